# revision 17
# baseline (speedup 1.0000x reference)
"""AdaLN cross-attention + FFN block on 8 TRN2 NeuronCores.

Sharding: 8 cores = 4 batches x 2 L-halves (512 rows each). No collectives:
K/V projections are duplicated across the pair of cores sharing a batch,
everything else splits cleanly along L.

Layout: the whole kernel runs TRANSPOSED — activations are [C, L] with the
channel dim on partitions. This makes every matmul natural (contraction dim
on partitions), makes the AdaLN scale/shift/gamma per-partition broadcasts,
and costs zero on-device transposes. The host supplies x^T, context^T,
exp(bias)^T (per-head [m, l]) and pre-transposed weights; the output comes
back as out^T and is transposed on host.

dtypes: the whole attention path (QKV projections, scores, attn*V, output
projection) runs in fp8-e4m3 with DoubleRow perf mode where the layout
allows (2 k-tiles contracted per pass = 2x PE throughput); weights are
pre-scaled by 16 on the host so fp8 values stay in the normal range, and the
inverse scales fold into activation-scale immediates. The FFN keeps bf16
matmuls (fp8 there pushes rel-err past the gate; measured by numpy
simulation). LayerNorm statistics and residuals stay f32.

The emission order is software-pipelined: V-projection runs first on PE
(needs only ctx+Wv DMA) while DVE/ACT stage the LN1 statistics, and the
attention loop runs with a two-iteration skew — scores for head-pair i,
attention*V for pair i-1, normalization for pair i-2 emitted together.
"""
import sys
if "/opt/trn_rl_repo" not in sys.path:
    sys.path.insert(0, "/opt/trn_rl_repo")

import numpy as np
import ml_dtypes

import concourse.bass as bass
import concourse.mybir as mybir
import concourse.tile as tile
from concourse import bacc
from concourse.bass_utils import run_bass_kernel_spmd

B, L, LC, C, H, HD = 4, 1024, 1024, 1024, 16, 64
P = 128
LH = 512                 # L rows per core
CT = C // P              # 8
MT = LC // P             # 8
E = 4 * C                # 4096
ET = E // P              # 32
SCALE = 0.25 / (HD ** 0.5)
EPS = 1e-5
WS = 16.0                # host-side fp8 weight scale (keeps values normal)
QSC = 1.0 / 32.0         # qT = q_psum * QSC when writing fp8
ESC = SCALE / (WS * WS * QSC)   # exp() scale: scores_psum -> true logits
OSC = 1.0 / (WS * WS)    # out-proj psum -> true attn@Wo

F32 = mybir.dt.float32
F32R = mybir.dt.float32r
BF16 = mybir.dt.bfloat16
F8 = mybir.dt.float8e4
AF = mybir.ActivationFunctionType
ALU = mybir.AluOpType
DR = mybir.MatmulPerfMode.DoubleRow

NCORES = 8


def build():
    nc = bacc.Bacc("TRN2", target_bir_lowering=False, debug=False, num_devices=NCORES)

    xT_d = nc.declare_dram_parameter("xT", [C, LH], F32, isOutput=False)
    xT8_d = nc.declare_dram_parameter("xT8", [C, LH], BF16, isOutput=False)
    ctxT_d = nc.declare_dram_parameter("ctxT", [C, LC], F8, isOutput=False)
    biasT_d = nc.declare_dram_parameter("biasT", [H, LC, LH], F8, isOutput=False)
    wqT_d = nc.declare_dram_parameter("wqT", [P, CT, CT, P], F8, isOutput=False)
    wkT_d = nc.declare_dram_parameter("wkT", [P, CT, CT, P], F8, isOutput=False)
    wvT_d = nc.declare_dram_parameter("wvT", [C, C], F8, isOutput=False)
    woT_d = nc.declare_dram_parameter("woT", [P, CT, CT, P], F8, isOutput=False)
    w1T_d = nc.declare_dram_parameter("w1T", [P, ET, CT, P], BF16, isOutput=False)
    w2T_d = nc.declare_dram_parameter("w2T", [P, CT, 2, 16, P], BF16, isOutput=False)
    bo_d = nc.declare_dram_parameter("bo", [P, CT], F32, isOutput=False)
    b1_d = nc.declare_dram_parameter("b1", [P, ET], F32, isOutput=False)
    b2_d = nc.declare_dram_parameter("b2", [P, CT], F32, isOutput=False)
    cond_d = nc.declare_dram_parameter("cond", [P, 6, CT], F32, isOutput=False)
    ada_d = nc.declare_dram_parameter("ada", [P, 6, CT], F32, isOutput=False)
    outT_d = nc.declare_dram_parameter("outT", [C, LH], F32, isOutput=True)

    with tile.TileContext(nc) as tc:
        with (
            tc.tile_pool(name="cn", bufs=1) as cn,
            tc.tile_pool(name="p1", bufs=1) as p1,
            tc.tile_pool(name="p2", bufs=2) as p2,
            tc.tile_pool(name="p3", bufs=3) as p3,
            tc.tile_pool(name="p4", bufs=4) as p4,
            tc.tile_pool(name="pp", bufs=4) as pp,
            tc.tile_pool(name="rw2", bufs=2) as rw2,
            tc.tile_pool(name="psS", bufs=3, space="PSUM") as psS,      # [P,1024] universal
            tc.tile_pool(name="psA", bufs=2, space="PSUM") as psA,      # [P,512] av/stats/bcast
        ):
            # ---------- constants / params ----------
            ones_f = cn.tile([P, 1], F32, tag="ones_f")
            nc.vector.memset(ones_f[:], 1.0)
            ones_col = cn.tile([P, 1], BF16, tag="ones_col")        # lhsT [K=P, M=1]
            nc.scalar.copy(ones_col[:], ones_f[:])
            ones_rf = cn.tile([1, P], F32, tag="ones_rf")
            nc.vector.memset(ones_rf[:], 1.0)
            ones_row = cn.tile([1, P], F32R, tag="ones_row")        # lhsT [K=1, M=P]
            nc.scalar.copy(ones_row[:], ones_rf[:])
            eps_t = cn.tile([1, 1], F32, tag="eps")
            nc.vector.memset(eps_t[:], EPS)

            # ---------- big persistent tensors ----------
            ctxT = p1.tile([P, CT, LC], F8, tag="ctx")
            wv = p1.tile([P, CT, C], F8, tag="wv")
            xT = p1.tile([P, CT, LH], F32, tag="xT")
            xT8 = p1.tile([P, CT, LH], BF16, tag="xT8")
            qT = p1.tile([P, CT, LH], F8, tag="qT")
            vaug = p1.tile([P, MT, H, HD + 1], F8, tag="vaug")
            nc.vector.memset(vaug[:, :, :, HD:HD + 1], 1.0)         # softmax-denominator ones column

            # bf16 xT tiles lead the queue so LN1 starts immediately (the f32
            # copy for the residual streams later, during attention);
            # ctx + Wv stream concurrently on other queues for the V matmuls
            for ct in range(CT):
                nc.sync.dma_start(xT8[:, ct, :], xT8_d[ct * P:(ct + 1) * P, :])
            nc.gpsimd.dma_start(ctxT[:], ctxT_d[:, :].rearrange("(o p) f -> p o f", p=P))
            nc.scalar.dma_start(wv[:], wvT_d[:, :].rearrange("(o p) d -> p o d", p=P))

            # ---------- AdaLN parameters ----------
            cond_t = cn.tile([P, 6, CT], F32, tag="cond")
            nc.sync.dma_start(cond_t[:], cond_d[:, :, :])
            ada_t = cn.tile([P, 6, CT], F32, tag="ada")
            nc.sync.dma_start(ada_t[:], ada_d[:, :, :])
            g_t = cn.tile([P, 6, CT], F32, tag="g")                 # gamma1,gamma2,scale1,scale2,shift1,shift2
            nc.vector.tensor_tensor(g_t[:], cond_t[:], ada_t[:], ALU.add)
            s1p1 = cn.tile([P, CT], F32, tag="s1p1")                # scale1 + 1
            nc.vector.tensor_scalar_add(s1p1[:], g_t[:, 2, :], 1.0)
            s2p1 = cn.tile([P, CT], F32, tag="s2p1")                # scale2 + 1
            nc.vector.tensor_scalar_add(s2p1[:], g_t[:, 3, :], 1.0)
            g1s = cn.tile([P, CT], F32, tag="g1s")                  # gamma1 / (WS*WS)
            nc.vector.tensor_scalar_mul(g1s[:], g_t[:, 0, :], OSC)
            bo_t = cn.tile([P, CT], F32, tag="bo")
            nc.sync.dma_start(bo_t[:], bo_d[:, :])
            b1_t = cn.tile([P, ET], F32, tag="b1")
            nc.sync.dma_start(b1_t[:], b1_d[:, :])
            b2_t = cn.tile([P, CT], F32, tag="b2")
            nc.sync.dma_start(b2_t[:], b2_d[:, :])
            bog1 = cn.tile([P, CT], F32, tag="bog1")                # bo * gamma1
            nc.vector.tensor_tensor(bog1[:], bo_t[:], g_t[:, 0, :], ALU.mult)
            b2g2 = cn.tile([P, CT], F32, tag="b2g2")                # b2 * gamma2
            nc.vector.tensor_tensor(b2g2[:], b2_t[:], g_t[:, 1, :], ALU.mult)

            # ---------- LN pieces ----------
            def ln_stage(src, ct):
                xr = p4.tile([P, LH], BF16, tag="tmpA")
                nc.vector.tensor_copy(xr[:], src[:, ct, :])
                sq = p4.tile([P, LH], BF16, tag="tmpB")
                nc.scalar.activation(sq[:], src[:, ct, :], AF.Square, bias=0.0, scale=1.0)
                return xr, sq

            def ln_mm(xsum, ssum, xr, sq, ct):
                nc.tensor.matmul(xsum[:], ones_col[:], xr if isinstance(xr, bass.AP) else xr[:],
                                 start=(ct == 0), stop=(ct == CT - 1))
                nc.tensor.matmul(ssum[:], ones_col[:], sq[:], start=(ct == 0), stop=(ct == CT - 1))

            def ln_rows(xsum, ssum):
                mu = cn.tile([1, LH], F32, tag="mu")
                nc.vector.tensor_scalar_mul(mu[:], xsum[:], 1.0 / C)
                ex2 = p3.tile([1, LH], F32, tag="rowtmp")
                nc.vector.tensor_scalar_mul(ex2[:], ssum[:], 1.0 / C)
                mu2 = p3.tile([1, LH], F32, tag="rowtmp")
                nc.vector.tensor_tensor(mu2[:], mu[:], mu[:], ALU.mult)
                var = p3.tile([1, LH], F32, tag="rowtmp")
                nc.vector.tensor_tensor(var[:], ex2[:], mu2[:], ALU.subtract)
                sd = p3.tile([1, LH], F32, tag="rowtmp")
                nc.scalar.activation(sd[:], var[:], AF.Sqrt, bias=eps_t[:, 0:1], scale=1.0)
                rstd = cn.tile([1, LH], F32, tag="rstd")
                nc.vector.reciprocal_approx_fast(rstd[:], sd[:])
                rstd_r = cn.tile([1, LH], F32R, tag="rstd_r")
                nc.scalar.copy(rstd_r[:], rstd[:])
                nmr = p3.tile([1, LH], F32, tag="rowtmp")
                nc.vector.tensor_tensor(nmr[:], mu[:], rstd[:], ALU.mult)
                nmr_r = cn.tile([1, LH], F32R, tag="nmr_r")
                nc.scalar.mul(nmr_r[:], nmr[:], -1.0)               # -(mu*rstd)
                return rstd_r, nmr_r

            def ln_bc(rstd_r, nmr_r):
                bc_rp = psA.tile([P, LH], F32, tag="avp")
                nc.tensor.matmul(bc_rp[:], ones_row[:], rstd_r[:], start=True, stop=True)
                bc_r = rw2.tile([P, LH], F32, tag="bcs")
                nc.scalar.copy(bc_r[:], bc_rp[:])
                bc_np = psA.tile([P, LH], F32, tag="avp")
                nc.tensor.matmul(bc_np[:], ones_row[:], nmr_r[:], start=True, stop=True)
                bc_n = rw2.tile([P, LH], F32, tag="bcs")
                nc.scalar.copy(bc_n[:], bc_np[:])
                return bc_r, bc_n

            def ln_apply(src, bc_r, bc_n, sc_col, sh_idx, out_mod):
                for ct in range(CT):
                    t1 = p4.tile([P, LH], F32, tag="tmpA")
                    nc.vector.tensor_tensor(t1[:], src[:, ct, :], bc_r[:], ALU.mult)
                    t2 = p4.tile([P, LH], F32, tag="tmpB")
                    nc.vector.tensor_tensor(t2[:], t1[:], bc_n[:], ALU.add)
                    nc.scalar.activation(out_mod[:, ct, :], t2[:], AF.Identity,
                                         bias=g_t[:, sh_idx, ct:ct + 1],
                                         scale=sc_col[:, ct:ct + 1])

            # ---------- V projection (fp8 DoubleRow over ct pairs) ----------
            def v_group(mt, half):
                v_ps = psS.tile([P, 512], F32, tag="accS")
                for i in range(CT // 2):
                    nc.tensor.matmul(v_ps[:], ctxT[:, 2 * i:2 * i + 2, mt * P:(mt + 1) * P],
                                     wv[:, 2 * i:2 * i + 2, half * 512:(half + 1) * 512],
                                     start=(i == 0), stop=(i == CT // 2 - 1), perf_mode=DR)
                nc.scalar.copy(vaug[:, mt, half * 8:(half + 1) * 8, 0:HD],
                               v_ps[:].rearrange("p (h d) -> p h d", d=HD))

            # LN1 stats read the bf16 xT8 tiles directly (x-sum matmul needs no
            # staging copy); ACT squares run while PE does the V projection
            xsum1 = psA.tile([1, LH], F32, tag="avp")
            ssum1 = psA.tile([1, LH], F32, tag="avp")
            sqs = []
            for ct in range(2):
                sq = p4.tile([P, LH], BF16, tag="tmpB")
                nc.scalar.activation(sq[:], xT8[:, ct, :], AF.Square, bias=0.0, scale=1.0)
                sqs.append(sq)
            for mt in range(MT):
                v_group(mt, 0)
                if mt < CT:
                    ln_mm(xsum1, ssum1, xT8[:, mt, :], sqs[mt % 2], mt)
                    if mt + 2 < CT:
                        sq = p4.tile([P, LH], BF16, tag="tmpB")
                        nc.scalar.activation(sq[:], xT8[:, mt + 2, :], AF.Square,
                                             bias=0.0, scale=1.0)
                        sqs[mt % 2] = sq
            r1, n1 = ln_rows(xsum1, ssum1)
            bc_r1, bc_n1 = ln_bc(r1, n1)
            for mt in range(MT):
                v_group(mt, 1)
            modx = p1.tile([P, CT, LH], F8, tag="mod8")
            ln_apply(xT8, bc_r1, bc_n1, s1p1, 4, modx)

            # ---------- K projection (fp8 DoubleRow) ----------
            def k_mm(dt):
                wk_st = p4.tile([P, CT, P], F8, tag="wst")
                nc.sync.dma_start(wk_st[:], wkT_d[:, dt, :, :])
                k_ps = psS.tile([P, LC], F32, tag="accS")
                for i in range(CT // 2):
                    nc.tensor.matmul(k_ps[:, 0:512], wk_st[:, 2 * i:2 * i + 2, :],
                                     ctxT[:, 2 * i:2 * i + 2, 0:512],
                                     start=(i == 0), stop=(i == CT // 2 - 1), perf_mode=DR)
                    nc.tensor.matmul(k_ps[:, 512:1024], wk_st[:, 2 * i:2 * i + 2, :],
                                     ctxT[:, 2 * i:2 * i + 2, 512:1024],
                                     start=(i == 0), stop=(i == CT // 2 - 1), perf_mode=DR)
                return k_ps

            def k_copy(k_ps):
                kdt = p2.tile([P, LC], F8, tag="kdt")
                nc.vector.tensor_copy(kdt[:], k_ps[:])
                return kdt

            # K0 runs on PE while ACT produces modx for the Q projection
            kd = {0: k_mm(0)}

            def q_proj(dt):
                wq_st = p4.tile([P, CT, P], F8, tag="wst")
                nc.sync.dma_start(wq_st[:], wqT_d[:, dt, :, :])
                q_ps = psS.tile([P, LH], F32, tag="accS")
                for i in range(CT // 2):
                    nc.tensor.matmul(q_ps[:], wq_st[:, 2 * i:2 * i + 2, :],
                                     modx[:, 2 * i:2 * i + 2, :],
                                     start=(i == 0), stop=(i == CT // 2 - 1), perf_mode=DR)
                nc.scalar.mul(qT[:, dt, :], q_ps[:], QSC)

            kd[0] = k_copy(kd[0])
            for dt in range(CT):
                q_proj(dt)
            # f32 x for the residual: streams during attention, needed ~150us in
            for ct in range(CT):
                nc.sync.dma_start(xT[:, ct, :], xT_d[ct * P:(ct + 1) * P, :])

            # ---------- attention: two-iteration software pipeline ----------
            # iteration i emits: finish(i-2) | k(i+1) | scores+softmax(i) | attnv(i-1)
            cat = p1.tile([P, CT, LH], F8, tag="cat")               # out^T of attention, head-concat

            def emit_scores(dt):
                kcur = kd[dt]
                out = []
                for hh in range(2):
                    h = 2 * dt + hh
                    probs = pp.tile([P, MT, LH], F8, tag="probs")
                    out.append(probs)
                    # exp(bias) host-precomputed (fp8), one DMA per head
                    bias_t = p3.tile([P, MT, LH], F8, tag="biast")
                    nc.sync.dma_start(
                        bias_t[:], biasT_d[h, :, :].rearrange("(t p) l -> p t l", p=P))
                    for mp in range(MT // 2):           # pairs of m-tiles
                        sc = psS.tile([P, LC], F32, tag="accS")
                        for j in range(2):
                            mt = 2 * mp + j
                            nc.tensor.matmul(
                                sc[:, j * 512:(j + 1) * 512],
                                kcur[hh * HD:(hh + 1) * HD, mt * P:(mt + 1) * P],
                                qT[hh * HD:(hh + 1) * HD, dt, :],
                                start=True, stop=True)
                        # exp(s)*exp(b): ACT does ONLY the exp (it paces the
                        # loop); the fp8 multiply splits between DVE and Pool
                        es = p4.tile([P, 2, LH], F8, tag="esb")
                        nc.scalar.activation(es[:], sc[:].rearrange("p (t l) -> p t l", t=2),
                                             AF.Exp, bias=0.0, scale=ESC)
                        eng = nc.vector if mp % 2 == 0 else nc.gpsimd
                        eng.tensor_tensor(probs[:, 2 * mp:2 * mp + 2, :], es[:],
                                          bias_t[:, 2 * mp:2 * mp + 2, :], ALU.mult)
                return out

            def emit_attnv(dt, probs2):
                out = []
                for hh in range(2):
                    h = 2 * dt + hh
                    probs = probs2[hh]
                    av = psA.tile([P, LH], F32, tag="avp")
                    out.append((dt, hh, av))
                    for mp in range(MT // 2):
                        nc.tensor.matmul(av[0:HD + 1, :], vaug[:, 2 * mp:2 * mp + 2, h, :],
                                         probs[:, 2 * mp:2 * mp + 2, :],
                                         start=(mp == 0), stop=(mp == MT // 2 - 1),
                                         perf_mode=DR)
                return out

            def emit_recip(pend_av):
                # DVE/Pool reciprocal chain (ACT-free: ACT paces the exp), at
                # the start of the NEXT iteration so it overlaps K projection
                out = []
                for (dt, hh, av) in pend_av:
                    ssr = p3.tile([1, LH], F32, tag="rowtmp")
                    nc.vector.tensor_copy(ssr[:], av[HD:HD + 1, :])
                    rec = p3.tile([1, LH], F32, tag="rowtmp")
                    nc.vector.reciprocal_approx_fast(rec[:], ssr[:])
                    rec_r = p3.tile([1, LH], F32R, tag="rowtmp")
                    nc.gpsimd.tensor_copy(rec_r[:], rec[:])
                    out.append((dt, hh, av, rec_r))
                return out

            def head_finish(pdt, phh, av, rec_r):
                bc_ps = psS.tile([P, LH], F32, tag="accS")
                nc.tensor.matmul(bc_ps[0:HD, :], ones_row[:, 0:HD], rec_r[:],
                                 start=True, stop=True)
                bc_s = p4.tile([HD, LH], F32, tag="tmpA")
                nc.vector.tensor_copy(bc_s[:], bc_ps[0:HD, :])
                if phh == 0:
                    nc.vector.tensor_tensor(cat[0:HD, pdt, :], av[0:HD, :], bc_s[:], ALU.mult)
                else:
                    tmp_o = p4.tile([HD, LH], F8, tag="tmpB")
                    nc.vector.tensor_tensor(tmp_o[:], av[0:HD, :], bc_s[:], ALU.mult)
                    nc.sync.dma_start(cat[HD:P, pdt, :], tmp_o[:])   # partition shift

            probs_by_dt = {}
            pend_fin = []   # (dt, hh, av, rec_r) for head_finish one iteration later
            for i in range(CT + 2):
                kp = None
                if i + 1 < CT:
                    kp = k_mm(i + 1)
                for args in pend_fin:
                    head_finish(*args)              # bc + normalization for pair i-1
                pend_fin = []
                if kp is not None:
                    kd[i + 1] = k_copy(kp)          # DVE copy before the probs mults
                av_out = []
                if i < CT:
                    probs_by_dt[i] = emit_scores(i)
                if 1 <= i <= CT:
                    av_out = emit_attnv(i - 1, probs_by_dt.pop(i - 1))
                pend_fin = emit_recip(av_out)       # DVE/ACT chain, runs early next iter
            for args in pend_fin:
                head_finish(*args)

            # ---------- output projection + gated residual, LN2 stats interleaved ----------
            xsum2 = psA.tile([1, LH], F32, tag="avp")
            ssum2 = psA.tile([1, LH], F32, tag="avp")
            for ct2 in range(CT):
                wo_st = p4.tile([P, CT, P], F8, tag="wst")
                nc.sync.dma_start(wo_st[:], woT_d[:, ct2, :, :])
                ao_ps = psS.tile([P, LH], F32, tag="accS")
                for i in range(CT // 2):
                    nc.tensor.matmul(ao_ps[:], wo_st[:, 2 * i:2 * i + 2, :],
                                     cat[:, 2 * i:2 * i + 2, :],
                                     start=(i == 0), stop=(i == CT // 2 - 1), perf_mode=DR)
                t = p4.tile([P, LH], F32, tag="tmpB")
                nc.scalar.activation(t[:], ao_ps[:], AF.Identity,
                                     bias=bog1[:, ct2:ct2 + 1], scale=g1s[:, ct2:ct2 + 1])
                nc.vector.tensor_tensor(xT[:, ct2, :], t[:], xT[:, ct2, :], ALU.add)
                xr, sq = ln_stage(xT, ct2)
                ln_mm(xsum2, ssum2, xr, sq, ct2)

            r2, n2 = ln_rows(xsum2, ssum2)
            bc_r2, bc_n2 = ln_bc(r2, n2)
            modf = p1.tile([P, CT, LH], BF16, tag="mod")
            ln_apply(xT, bc_r2, bc_n2, s2p1, 5, modf)

            # ---------- FFN (bf16: fp8 fails the accuracy gate here) ----------
            hT = p1.tile([P, ET, LH], BF16, tag="bigA")             # reuses ctxT slot
            for et in range(ET):
                w1_st = p4.tile([P, CT, P], BF16, tag="wst")
                (nc.sync if et % 2 == 0 else nc.gpsimd).dma_start(w1_st[:], w1T_d[:, et, :, :])
                h_ps = psS.tile([P, LH], F32, tag="accS")
                for ct in range(CT):
                    nc.tensor.matmul(h_ps[:], w1_st[:, ct, :], modf[:, ct, :],
                                     start=(ct == 0), stop=(ct == CT - 1))
                nc.scalar.activation(hT[:, et, :], h_ps[:], AF.Gelu_apprx_tanh,
                                     bias=b1_t[:, et:et + 1], scale=1.0)

            for ct2 in range(CT):
                f_ps = psS.tile([P, LH], F32, tag="accS")
                for eh in range(2):
                    w2_st = p2.tile([P, 16, P], BF16, tag="w2st")
                    (nc.sync if eh == 0 else nc.gpsimd).dma_start(w2_st[:], w2T_d[:, ct2, eh, :, :])
                    for ei in range(16):
                        et = eh * 16 + ei
                        nc.tensor.matmul(f_ps[:], w2_st[:, ei, :], hT[:, et, :],
                                         start=(et == 0), stop=(et == ET - 1))
                t = p4.tile([P, LH], F32, tag="tmpB")
                nc.scalar.activation(t[:], f_ps[:], AF.Identity,
                                     bias=b2g2[:, ct2:ct2 + 1], scale=g_t[:, 1, ct2:ct2 + 1])
                o_t = p4.tile([P, LH], F32, tag="tmpA")
                nc.vector.tensor_tensor(o_t[:], t[:], xT[:, ct2, :], ALU.add)
                nc.sync.dma_start(outT_d[ct2 * P:(ct2 + 1) * P, :], o_t[:])

    nc.compile()
    return nc


_NC = None


def _get_nc():
    global _NC
    if _NC is None:
        _NC = build()
    return _NC


def _shard(inputs):
    f32 = lambda a: np.ascontiguousarray(a, dtype=np.float32)
    bf16 = ml_dtypes.bfloat16
    f8 = ml_dtypes.float8_e4m3
    x = f32(inputs["x"]); context = f32(inputs["context"])
    cond_BD = f32(inputs["cond_BD"]); attn_bias = f32(inputs["attn_bias"])
    ada_gss = f32(inputs["ada_gss"])
    Wq = f32(inputs["Wq"]); Wk = f32(inputs["Wk"]); Wv = f32(inputs["Wv"])
    Wo = f32(inputs["Wo"]); bo = f32(inputs["bo"])
    W1 = f32(inputs["W1"]); b1 = f32(inputs["b1"])
    W2 = f32(inputs["W2"]); b2 = f32(inputs["b2"])

    shared = {
        "wqT": np.ascontiguousarray(
            (WS * Wq).T.reshape(CT, P, CT, P).transpose(1, 2, 0, 3)).astype(f8),
        "wkT": np.ascontiguousarray(
            (WS * Wk).T.reshape(CT, P, CT, P).transpose(1, 2, 0, 3)).astype(f8),
        "wvT": np.ascontiguousarray((WS * Wv).T).astype(f8),
        "woT": np.ascontiguousarray(
            (WS * Wo).T.reshape(CT, P, CT, P).transpose(1, 2, 0, 3)).astype(f8),
        "w1T": np.ascontiguousarray(
            W1.T.reshape(CT, P, ET, P).transpose(1, 2, 0, 3)).astype(bf16),
        "w2T": np.ascontiguousarray(
            W2.T.reshape(2, 16, P, CT, P).transpose(2, 3, 0, 1, 4)).astype(bf16),
        "bo": np.ascontiguousarray(bo.reshape(CT, P).T),
        "b1": np.ascontiguousarray(b1.reshape(ET, P).T),
        "b2": np.ascontiguousarray(b2.reshape(CT, P).T),
        "ada": np.ascontiguousarray(ada_gss[0, 0].reshape(6, CT, P).transpose(2, 0, 1)),
    }
    in_maps = []
    for i in range(NCORES):
        b, lh = i // 2, i % 2
        l0 = lh * LH
        m = dict(shared)
        m["xT"] = np.ascontiguousarray(x[b, l0:l0 + LH, :].T)
        m["xT8"] = np.ascontiguousarray(x[b, l0:l0 + LH, :].T).astype(bf16)
        m["ctxT"] = np.ascontiguousarray(context[b].T).astype(f8)
        m["biasT"] = np.exp(np.ascontiguousarray(
            attn_bias[b, :, l0:l0 + LH, :].transpose(0, 2, 1))).astype(f8)
        m["cond"] = np.ascontiguousarray(cond_BD[b, 0].reshape(6, CT, P).transpose(2, 0, 1))
        in_maps.append(m)
    return in_maps


def kernel(**inputs) -> np.ndarray:
    nc = _get_nc()
    in_maps = _shard(inputs)
    res = run_bass_kernel_spmd(nc, in_maps, core_ids=list(range(NCORES)))
    out = np.empty((B, L, C), dtype=np.float32)
    for i in range(NCORES):
        b, lh = i // 2, i % 2
        out[b, lh * LH:(lh + 1) * LH, :] = res.results[i]["outT"].T
    return out


# revision 20
# speedup vs baseline: 1.0143x; 1.0143x over previous
"""AdaLN cross-attention + FFN block on 8 TRN2 NeuronCores.

Sharding: 8 cores = 4 batches x 2 L-halves (512 rows each). No collectives:
K/V projections are duplicated across the pair of cores sharing a batch,
everything else splits cleanly along L.

Layout: the whole kernel runs TRANSPOSED — activations are [C, L] with the
channel dim on partitions. This makes every matmul natural (contraction dim
on partitions), makes the AdaLN scale/shift/gamma per-partition broadcasts,
and costs zero on-device transposes. The host supplies x^T, context^T,
exp(bias)^T (per-head [m, l]) and pre-transposed weights; the output comes
back as out^T and is transposed on host.

dtypes: the whole attention path (QKV projections, scores, attn*V, output
projection) runs in fp8-e4m3 with DoubleRow perf mode where the layout
allows (2 k-tiles contracted per pass = 2x PE throughput); weights are
pre-scaled by 16 on the host so fp8 values stay in the normal range, and the
inverse scales fold into activation-scale immediates. The FFN keeps bf16
matmuls (fp8 there pushes rel-err past the gate; measured by numpy
simulation). LayerNorm statistics and residuals stay f32.

The emission order is software-pipelined: V-projection runs first on PE
(needs only ctx+Wv DMA) while DVE/ACT stage the LN1 statistics, and the
attention loop runs with a two-iteration skew — scores for head-pair i,
attention*V for pair i-1, normalization for pair i-2 emitted together.
"""
import sys
if "/opt/trn_rl_repo" not in sys.path:
    sys.path.insert(0, "/opt/trn_rl_repo")

import numpy as np
import ml_dtypes

import concourse.bass as bass
import concourse.mybir as mybir
import concourse.tile as tile
from concourse import bacc
from concourse.bass_utils import run_bass_kernel_spmd

B, L, LC, C, H, HD = 4, 1024, 1024, 1024, 16, 64
P = 128
LH = 512                 # L rows per core
CT = C // P              # 8
MT = LC // P             # 8
E = 4 * C                # 4096
ET = E // P              # 32
SCALE = 0.25 / (HD ** 0.5)
EPS = 1e-5
WS = 16.0                # host-side fp8 weight scale (keeps values normal)
QSC = 1.0 / 32.0         # qT = q_psum * QSC when writing fp8
ESC = SCALE / (WS * WS * QSC)   # exp() scale: scores_psum -> true logits
OSC = 1.0 / (WS * WS)    # out-proj psum -> true attn@Wo

F32 = mybir.dt.float32
F32R = mybir.dt.float32r
BF16 = mybir.dt.bfloat16
F8 = mybir.dt.float8e4
AF = mybir.ActivationFunctionType
ALU = mybir.AluOpType
DR = mybir.MatmulPerfMode.DoubleRow

NCORES = 8


def build():
    nc = bacc.Bacc("TRN2", target_bir_lowering=False, debug=False, num_devices=NCORES)

    xT_d = nc.declare_dram_parameter("xT", [C, LH], F32, isOutput=False)
    xT8_d = nc.declare_dram_parameter("xT8", [C, LH], BF16, isOutput=False)
    ctxT_d = nc.declare_dram_parameter("ctxT", [C, LC], F8, isOutput=False)
    biasT_d = nc.declare_dram_parameter("biasT", [H, LC, LH], F8, isOutput=False)
    wqT_d = nc.declare_dram_parameter("wqT", [P, CT, CT, P], F8, isOutput=False)
    wkT_d = nc.declare_dram_parameter("wkT", [P, CT, CT, P], F8, isOutput=False)
    wvT_d = nc.declare_dram_parameter("wvT", [C, C], F8, isOutput=False)
    woT_d = nc.declare_dram_parameter("woT", [P, CT, CT, P], F8, isOutput=False)
    w1T_d = nc.declare_dram_parameter("w1T", [P, ET, CT, P], BF16, isOutput=False)
    w2T_d = nc.declare_dram_parameter("w2T", [P, CT, 2, 16, P], BF16, isOutput=False)
    bo_d = nc.declare_dram_parameter("bo", [P, CT], F32, isOutput=False)
    b1_d = nc.declare_dram_parameter("b1", [P, ET], F32, isOutput=False)
    b2_d = nc.declare_dram_parameter("b2", [P, CT], F32, isOutput=False)
    cond_d = nc.declare_dram_parameter("cond", [P, 6, CT], F32, isOutput=False)
    ada_d = nc.declare_dram_parameter("ada", [P, 6, CT], F32, isOutput=False)
    outT_d = nc.declare_dram_parameter("outT", [C, LH], F32, isOutput=True)

    with tile.TileContext(nc) as tc:
        with (
            tc.tile_pool(name="cn", bufs=1) as cn,
            tc.tile_pool(name="p1", bufs=1) as p1,
            tc.tile_pool(name="p2", bufs=2) as p2,
            tc.tile_pool(name="p3", bufs=3) as p3,
            tc.tile_pool(name="p4", bufs=4) as p4,
            tc.tile_pool(name="pp", bufs=4) as pp,
            tc.tile_pool(name="rw2", bufs=2) as rw2,
            tc.tile_pool(name="psS", bufs=3, space="PSUM") as psS,      # [P,1024] universal
            tc.tile_pool(name="psA", bufs=2, space="PSUM") as psA,      # [P,512] av/stats/bcast
        ):
            # ---------- constants / params ----------
            ones_f = cn.tile([P, 1], F32, tag="ones_f")
            nc.vector.memset(ones_f[:], 1.0)
            ones_col = cn.tile([P, 1], BF16, tag="ones_col")        # lhsT [K=P, M=1]
            nc.scalar.copy(ones_col[:], ones_f[:])
            ones_rf = cn.tile([1, P], F32, tag="ones_rf")
            nc.vector.memset(ones_rf[:], 1.0)
            ones_row = cn.tile([1, P], F32R, tag="ones_row")        # lhsT [K=1, M=P]
            nc.scalar.copy(ones_row[:], ones_rf[:])
            eps_t = cn.tile([1, 1], F32, tag="eps")
            nc.vector.memset(eps_t[:], EPS)

            # ---------- big persistent tensors ----------
            ctxT = p1.tile([P, CT, LC], F8, tag="ctx")
            wv = p1.tile([P, CT, C], F8, tag="wv")
            xT = p1.tile([P, CT, LH], F32, tag="xT")
            xT8 = p1.tile([P, CT, LH], BF16, tag="xT8")
            qT = p1.tile([P, CT, LH], F8, tag="qT")
            vaug = p1.tile([P, MT, H, HD + 1], F8, tag="vaug")
            nc.vector.memset(vaug[:, :, :, HD:HD + 1], 1.0)         # softmax-denominator ones column

            # bf16 xT tiles lead the queue so LN1 starts immediately (the f32
            # copy for the residual streams later, during attention);
            # ctx + Wv stream concurrently on other queues for the V matmuls
            for ct in range(CT):
                nc.sync.dma_start(xT8[:, ct, :], xT8_d[ct * P:(ct + 1) * P, :])
            nc.gpsimd.dma_start(ctxT[:], ctxT_d[:, :].rearrange("(o p) f -> p o f", p=P))
            nc.scalar.dma_start(wv[:], wvT_d[:, :].rearrange("(o p) d -> p o d", p=P))

            # ---------- AdaLN parameters ----------
            cond_t = cn.tile([P, 6, CT], F32, tag="cond")
            nc.sync.dma_start(cond_t[:], cond_d[:, :, :])
            ada_t = cn.tile([P, 6, CT], F32, tag="ada")
            nc.sync.dma_start(ada_t[:], ada_d[:, :, :])
            g_t = cn.tile([P, 6, CT], F32, tag="g")                 # gamma1,gamma2,scale1,scale2,shift1,shift2
            nc.vector.tensor_tensor(g_t[:], cond_t[:], ada_t[:], ALU.add)
            s1p1 = cn.tile([P, CT], F32, tag="s1p1")                # scale1 + 1
            nc.vector.tensor_scalar_add(s1p1[:], g_t[:, 2, :], 1.0)
            s2p1 = cn.tile([P, CT], F32, tag="s2p1")                # scale2 + 1
            nc.vector.tensor_scalar_add(s2p1[:], g_t[:, 3, :], 1.0)
            g1s = cn.tile([P, CT], F32, tag="g1s")                  # gamma1 / (WS*WS)
            nc.vector.tensor_scalar_mul(g1s[:], g_t[:, 0, :], OSC)
            bo_t = cn.tile([P, CT], F32, tag="bo")
            nc.sync.dma_start(bo_t[:], bo_d[:, :])
            b1_t = cn.tile([P, ET], F32, tag="b1")
            nc.sync.dma_start(b1_t[:], b1_d[:, :])
            b2_t = cn.tile([P, CT], F32, tag="b2")
            nc.sync.dma_start(b2_t[:], b2_d[:, :])
            bog1 = cn.tile([P, CT], F32, tag="bog1")                # bo * gamma1
            nc.vector.tensor_tensor(bog1[:], bo_t[:], g_t[:, 0, :], ALU.mult)
            b2g2 = cn.tile([P, CT], F32, tag="b2g2")                # b2 * gamma2
            nc.vector.tensor_tensor(b2g2[:], b2_t[:], g_t[:, 1, :], ALU.mult)

            # ---------- LN pieces ----------
            def ln_stage(src, ct):
                xr = p4.tile([P, LH], BF16, tag="tmpA")
                nc.vector.tensor_copy(xr[:], src[:, ct, :])
                sq = p4.tile([P, LH], BF16, tag="tmpB")
                nc.scalar.activation(sq[:], src[:, ct, :], AF.Square, bias=0.0, scale=1.0)
                return xr, sq

            def ln_mm(xsum, ssum, xr, sq, ct):
                nc.tensor.matmul(xsum[:], ones_col[:], xr if isinstance(xr, bass.AP) else xr[:],
                                 start=(ct == 0), stop=(ct == CT - 1))
                nc.tensor.matmul(ssum[:], ones_col[:], sq[:], start=(ct == 0), stop=(ct == CT - 1))

            def ln_rows(xsum, ssum):
                mu = cn.tile([1, LH], F32, tag="mu")
                nc.vector.tensor_scalar_mul(mu[:], xsum[:], 1.0 / C)
                ex2 = p3.tile([1, LH], F32, tag="rowtmp")
                nc.vector.tensor_scalar_mul(ex2[:], ssum[:], 1.0 / C)
                mu2 = p3.tile([1, LH], F32, tag="rowtmp")
                nc.vector.tensor_tensor(mu2[:], mu[:], mu[:], ALU.mult)
                var = p3.tile([1, LH], F32, tag="rowtmp")
                nc.vector.tensor_tensor(var[:], ex2[:], mu2[:], ALU.subtract)
                sd = p3.tile([1, LH], F32, tag="rowtmp")
                nc.scalar.activation(sd[:], var[:], AF.Sqrt, bias=eps_t[:, 0:1], scale=1.0)
                rstd = cn.tile([1, LH], F32, tag="rstd")
                nc.vector.reciprocal_approx_fast(rstd[:], sd[:])
                rstd_r = cn.tile([1, LH], F32R, tag="rstd_r")
                nc.scalar.copy(rstd_r[:], rstd[:])
                nmr = p3.tile([1, LH], F32, tag="rowtmp")
                nc.vector.tensor_tensor(nmr[:], mu[:], rstd[:], ALU.mult)
                nmr_r = cn.tile([1, LH], F32R, tag="nmr_r")
                nc.scalar.mul(nmr_r[:], nmr[:], -1.0)               # -(mu*rstd)
                return rstd_r, nmr_r

            def ln_bc(rstd_r, nmr_r):
                bc_rp = psA.tile([P, LH], F32, tag="avp")
                nc.tensor.matmul(bc_rp[:], ones_row[:], rstd_r[:], start=True, stop=True)
                bc_r = rw2.tile([P, LH], F32, tag="bcs")
                nc.scalar.copy(bc_r[:], bc_rp[:])
                bc_np = psA.tile([P, LH], F32, tag="avp")
                nc.tensor.matmul(bc_np[:], ones_row[:], nmr_r[:], start=True, stop=True)
                bc_n = rw2.tile([P, LH], F32, tag="bcs")
                nc.scalar.copy(bc_n[:], bc_np[:])
                return bc_r, bc_n

            def ln_apply(src, bc_r, bc_n, sc_col, sh_idx, out_mod):
                for ct in range(CT):
                    t1 = p4.tile([P, LH], F32, tag="tmpA")
                    nc.vector.tensor_tensor(t1[:], src[:, ct, :], bc_r[:], ALU.mult)
                    t2 = p4.tile([P, LH], F32, tag="tmpB")
                    nc.vector.tensor_tensor(t2[:], t1[:], bc_n[:], ALU.add)
                    nc.scalar.activation(out_mod[:, ct, :], t2[:], AF.Identity,
                                         bias=g_t[:, sh_idx, ct:ct + 1],
                                         scale=sc_col[:, ct:ct + 1])

            # ---------- V projection (fp8 DoubleRow over ct pairs) ----------
            def v_group(mt, half):
                v_ps = psS.tile([P, 512], F32, tag="accS")
                for i in range(CT // 2):
                    nc.tensor.matmul(v_ps[:], ctxT[:, 2 * i:2 * i + 2, mt * P:(mt + 1) * P],
                                     wv[:, 2 * i:2 * i + 2, half * 512:(half + 1) * 512],
                                     start=(i == 0), stop=(i == CT // 2 - 1), perf_mode=DR)
                nc.scalar.copy(vaug[:, mt, half * 8:(half + 1) * 8, 0:HD],
                               v_ps[:].rearrange("p (h d) -> p h d", d=HD))

            # LN1 stats read the bf16 xT8 tiles directly (x-sum matmul needs no
            # staging copy); ACT squares run while PE does the V projection
            xsum1 = psA.tile([1, LH], F32, tag="avp")
            ssum1 = psA.tile([1, LH], F32, tag="avp")
            sqs = []
            for ct in range(2):
                sq = p4.tile([P, LH], BF16, tag="tmpB")
                nc.scalar.activation(sq[:], xT8[:, ct, :], AF.Square, bias=0.0, scale=1.0)
                sqs.append(sq)
            for mt in range(MT):
                v_group(mt, 0)
                if mt < CT:
                    ln_mm(xsum1, ssum1, xT8[:, mt, :], sqs[mt % 2], mt)
                    if mt + 2 < CT:
                        sq = p4.tile([P, LH], BF16, tag="tmpB")
                        nc.scalar.activation(sq[:], xT8[:, mt + 2, :], AF.Square,
                                             bias=0.0, scale=1.0)
                        sqs[mt % 2] = sq
            r1, n1 = ln_rows(xsum1, ssum1)
            bc_r1, bc_n1 = ln_bc(r1, n1)
            for mt in range(MT):
                v_group(mt, 1)
            modx = p1.tile([P, CT, LH], F8, tag="mod8")
            ln_apply(xT8, bc_r1, bc_n1, s1p1, 4, modx)

            # ---------- K projection (fp8 DoubleRow) ----------
            def k_mm(dt):
                wk_st = p4.tile([P, CT, P], F8, tag="wst")
                nc.sync.dma_start(wk_st[:], wkT_d[:, dt, :, :])
                k_ps = psS.tile([P, LC], F32, tag="accS")
                for i in range(CT // 2):
                    nc.tensor.matmul(k_ps[:, 0:512], wk_st[:, 2 * i:2 * i + 2, :],
                                     ctxT[:, 2 * i:2 * i + 2, 0:512],
                                     start=(i == 0), stop=(i == CT // 2 - 1), perf_mode=DR)
                    nc.tensor.matmul(k_ps[:, 512:1024], wk_st[:, 2 * i:2 * i + 2, :],
                                     ctxT[:, 2 * i:2 * i + 2, 512:1024],
                                     start=(i == 0), stop=(i == CT // 2 - 1), perf_mode=DR)
                return k_ps

            def k_copy(k_ps):
                kdt = p2.tile([P, LC], F8, tag="kdt")
                nc.vector.tensor_copy(kdt[:], k_ps[:])
                return kdt

            # K0 runs on PE while ACT produces modx for the Q projection
            kd = {0: k_mm(0)}

            def q_proj(dt):
                wq_st = p4.tile([P, CT, P], F8, tag="wst")
                nc.sync.dma_start(wq_st[:], wqT_d[:, dt, :, :])
                q_ps = psS.tile([P, LH], F32, tag="accS")
                for i in range(CT // 2):
                    nc.tensor.matmul(q_ps[:], wq_st[:, 2 * i:2 * i + 2, :],
                                     modx[:, 2 * i:2 * i + 2, :],
                                     start=(i == 0), stop=(i == CT // 2 - 1), perf_mode=DR)
                nc.scalar.mul(qT[:, dt, :], q_ps[:], QSC)

            kd[0] = k_copy(kd[0])
            for dt in range(CT):
                q_proj(dt)
            # f32 x for the residual: streams during attention, needed ~150us in
            for ct in range(CT):
                nc.sync.dma_start(xT[:, ct, :], xT_d[ct * P:(ct + 1) * P, :])

            # ---------- attention: two-iteration software pipeline ----------
            # iteration i emits: finish(i-2) | k(i+1) | scores+softmax(i) | attnv(i-1)
            cat = p1.tile([P, CT, LH], F8, tag="cat")               # out^T of attention, head-concat

            def emit_scores(dt):
                kcur = kd[dt]
                out = []
                for hh in range(2):
                    h = 2 * dt + hh
                    probs = pp.tile([P, MT, LH], F8, tag="probs")
                    out.append(probs)
                    # exp(bias) host-precomputed (fp8), one DMA per head
                    bias_t = p3.tile([P, MT, LH], F8, tag="biast")
                    nc.sync.dma_start(
                        bias_t[:], biasT_d[h, :, :].rearrange("(t p) l -> p t l", p=P))
                    for mp in range(MT // 2):           # pairs of m-tiles
                        sc = psS.tile([P, LC], F32, tag="accS")
                        for j in range(2):
                            mt = 2 * mp + j
                            nc.tensor.matmul(
                                sc[:, j * 512:(j + 1) * 512],
                                kcur[hh * HD:(hh + 1) * HD, mt * P:(mt + 1) * P],
                                qT[hh * HD:(hh + 1) * HD, dt, :],
                                start=True, stop=True)
                        # exp(s)*exp(b): ACT does ONLY the exp (it paces the
                        # loop); the fp8 multiply splits between DVE and Pool
                        es = p4.tile([P, 2, LH], F8, tag="esb")
                        nc.scalar.activation(es[:], sc[:].rearrange("p (t l) -> p t l", t=2),
                                             AF.Exp, bias=0.0, scale=ESC)
                        nc.vector.tensor_tensor(probs[:, 2 * mp:2 * mp + 2, :], es[:],
                                                bias_t[:, 2 * mp:2 * mp + 2, :], ALU.mult)
                return out

            def emit_attnv(dt, probs2):
                out = []
                for hh in range(2):
                    h = 2 * dt + hh
                    probs = probs2[hh]
                    av = psA.tile([P, LH], F32, tag="avp")
                    out.append((dt, hh, av))
                    for mp in range(MT // 2):
                        nc.tensor.matmul(av[0:HD + 1, :], vaug[:, 2 * mp:2 * mp + 2, h, :],
                                         probs[:, 2 * mp:2 * mp + 2, :],
                                         start=(mp == 0), stop=(mp == MT // 2 - 1),
                                         perf_mode=DR)
                return out

            def emit_recip(pend_av):
                # DVE/Pool reciprocal chain (ACT-free: ACT paces the exp), at
                # the start of the NEXT iteration so it overlaps K projection
                out = []
                for (dt, hh, av) in pend_av:
                    ssr = p3.tile([1, LH], F32, tag="rowtmp")
                    nc.vector.tensor_copy(ssr[:], av[HD:HD + 1, :])
                    rec = p3.tile([1, LH], F32, tag="rowtmp")
                    nc.vector.reciprocal_approx_fast(rec[:], ssr[:])
                    rec_r = p3.tile([1, LH], F32R, tag="rowtmp")
                    nc.vector.tensor_copy(rec_r[:], rec[:])
                    out.append((dt, hh, av, rec_r))
                return out

            def head_finish(pdt, phh, av, rec_r):
                bc_ps = psS.tile([P, LH], F32, tag="accS")
                nc.tensor.matmul(bc_ps[0:HD, :], ones_row[:, 0:HD], rec_r[:],
                                 start=True, stop=True)
                bc_s = p4.tile([HD, LH], F32, tag="tmpA")
                nc.scalar.copy(bc_s[:], bc_ps[0:HD, :])
                if phh == 0:
                    nc.vector.tensor_tensor(cat[0:HD, pdt, :], av[0:HD, :], bc_s[:], ALU.mult)
                else:
                    tmp_o = p4.tile([HD, LH], F8, tag="tmpB")
                    nc.vector.tensor_tensor(tmp_o[:], av[0:HD, :], bc_s[:], ALU.mult)
                    nc.sync.dma_start(cat[HD:P, pdt, :], tmp_o[:])   # partition shift

            probs_by_dt = {}
            pend_fin = []   # (dt, hh, av, rec_r) for head_finish one iteration later
            for i in range(CT + 2):
                kp = None
                if i + 1 < CT:
                    kp = k_mm(i + 1)
                for args in pend_fin:
                    head_finish(*args)              # bc + normalization for pair i-1
                pend_fin = []
                if kp is not None:
                    kd[i + 1] = k_copy(kp)          # DVE copy before the probs mults
                av_out = []
                if i < CT:
                    probs_by_dt[i] = emit_scores(i)
                if 1 <= i <= CT:
                    av_out = emit_attnv(i - 1, probs_by_dt.pop(i - 1))
                pend_fin = emit_recip(av_out)       # DVE/ACT chain, runs early next iter
            for args in pend_fin:
                head_finish(*args)

            # ---------- output projection + gated residual, LN2 stats interleaved ----------
            xsum2 = psA.tile([1, LH], F32, tag="avp")
            ssum2 = psA.tile([1, LH], F32, tag="avp")
            for ct2 in range(CT):
                wo_st = p4.tile([P, CT, P], F8, tag="wst")
                nc.sync.dma_start(wo_st[:], woT_d[:, ct2, :, :])
                ao_ps = psS.tile([P, LH], F32, tag="accS")
                for i in range(CT // 2):
                    nc.tensor.matmul(ao_ps[:], wo_st[:, 2 * i:2 * i + 2, :],
                                     cat[:, 2 * i:2 * i + 2, :],
                                     start=(i == 0), stop=(i == CT // 2 - 1), perf_mode=DR)
                t = p4.tile([P, LH], F32, tag="tmpB")
                nc.scalar.activation(t[:], ao_ps[:], AF.Identity,
                                     bias=bog1[:, ct2:ct2 + 1], scale=g1s[:, ct2:ct2 + 1])
                nc.vector.tensor_tensor(xT[:, ct2, :], t[:], xT[:, ct2, :], ALU.add)
                xr, sq = ln_stage(xT, ct2)
                ln_mm(xsum2, ssum2, xr, sq, ct2)

            r2, n2 = ln_rows(xsum2, ssum2)
            bc_r2, bc_n2 = ln_bc(r2, n2)
            modf = p1.tile([P, CT, LH], BF16, tag="mod")
            ln_apply(xT, bc_r2, bc_n2, s2p1, 5, modf)

            # ---------- FFN (bf16: fp8 fails the accuracy gate here) ----------
            hT = p1.tile([P, ET, LH], BF16, tag="bigA")             # reuses ctxT slot
            for et in range(ET):
                w1_st = p4.tile([P, CT, P], BF16, tag="wst")
                (nc.sync if et % 2 == 0 else nc.gpsimd).dma_start(w1_st[:], w1T_d[:, et, :, :])
                h_ps = psS.tile([P, LH], F32, tag="accS")
                for ct in range(CT):
                    nc.tensor.matmul(h_ps[:], w1_st[:, ct, :], modf[:, ct, :],
                                     start=(ct == 0), stop=(ct == CT - 1))
                nc.scalar.activation(hT[:, et, :], h_ps[:], AF.Gelu_apprx_tanh,
                                     bias=b1_t[:, et:et + 1], scale=1.0)

            for ct2 in range(CT):
                f_ps = psS.tile([P, LH], F32, tag="accS")
                for eh in range(2):
                    w2_st = p2.tile([P, 16, P], BF16, tag="w2st")
                    (nc.sync if eh == 0 else nc.gpsimd).dma_start(w2_st[:], w2T_d[:, ct2, eh, :, :])
                    for ei in range(16):
                        et = eh * 16 + ei
                        nc.tensor.matmul(f_ps[:], w2_st[:, ei, :], hT[:, et, :],
                                         start=(et == 0), stop=(et == ET - 1))
                t = p4.tile([P, LH], F32, tag="tmpB")
                nc.scalar.activation(t[:], f_ps[:], AF.Identity,
                                     bias=b2g2[:, ct2:ct2 + 1], scale=g_t[:, 1, ct2:ct2 + 1])
                o_t = p4.tile([P, LH], F32, tag="tmpA")
                nc.vector.tensor_tensor(o_t[:], t[:], xT[:, ct2, :], ALU.add)
                nc.sync.dma_start(outT_d[ct2 * P:(ct2 + 1) * P, :], o_t[:])

    nc.compile()
    return nc


_NC = None


def _get_nc():
    global _NC
    if _NC is None:
        _NC = build()
    return _NC


def _shard(inputs):
    f32 = lambda a: np.ascontiguousarray(a, dtype=np.float32)
    bf16 = ml_dtypes.bfloat16
    f8 = ml_dtypes.float8_e4m3
    x = f32(inputs["x"]); context = f32(inputs["context"])
    cond_BD = f32(inputs["cond_BD"]); attn_bias = f32(inputs["attn_bias"])
    ada_gss = f32(inputs["ada_gss"])
    Wq = f32(inputs["Wq"]); Wk = f32(inputs["Wk"]); Wv = f32(inputs["Wv"])
    Wo = f32(inputs["Wo"]); bo = f32(inputs["bo"])
    W1 = f32(inputs["W1"]); b1 = f32(inputs["b1"])
    W2 = f32(inputs["W2"]); b2 = f32(inputs["b2"])

    shared = {
        "wqT": np.ascontiguousarray(
            (WS * Wq).T.reshape(CT, P, CT, P).transpose(1, 2, 0, 3)).astype(f8),
        "wkT": np.ascontiguousarray(
            (WS * Wk).T.reshape(CT, P, CT, P).transpose(1, 2, 0, 3)).astype(f8),
        "wvT": np.ascontiguousarray((WS * Wv).T).astype(f8),
        "woT": np.ascontiguousarray(
            (WS * Wo).T.reshape(CT, P, CT, P).transpose(1, 2, 0, 3)).astype(f8),
        "w1T": np.ascontiguousarray(
            W1.T.reshape(CT, P, ET, P).transpose(1, 2, 0, 3)).astype(bf16),
        "w2T": np.ascontiguousarray(
            W2.T.reshape(2, 16, P, CT, P).transpose(2, 3, 0, 1, 4)).astype(bf16),
        "bo": np.ascontiguousarray(bo.reshape(CT, P).T),
        "b1": np.ascontiguousarray(b1.reshape(ET, P).T),
        "b2": np.ascontiguousarray(b2.reshape(CT, P).T),
        "ada": np.ascontiguousarray(ada_gss[0, 0].reshape(6, CT, P).transpose(2, 0, 1)),
    }
    in_maps = []
    for i in range(NCORES):
        b, lh = i // 2, i % 2
        l0 = lh * LH
        m = dict(shared)
        m["xT"] = np.ascontiguousarray(x[b, l0:l0 + LH, :].T)
        m["xT8"] = np.ascontiguousarray(x[b, l0:l0 + LH, :].T).astype(bf16)
        m["ctxT"] = np.ascontiguousarray(context[b].T).astype(f8)
        m["biasT"] = np.exp(np.ascontiguousarray(
            attn_bias[b, :, l0:l0 + LH, :].transpose(0, 2, 1))).astype(f8)
        m["cond"] = np.ascontiguousarray(cond_BD[b, 0].reshape(6, CT, P).transpose(2, 0, 1))
        in_maps.append(m)
    return in_maps


def kernel(**inputs) -> np.ndarray:
    nc = _get_nc()
    in_maps = _shard(inputs)
    res = run_bass_kernel_spmd(nc, in_maps, core_ids=list(range(NCORES)))
    out = np.empty((B, L, C), dtype=np.float32)
    for i in range(NCORES):
        b, lh = i // 2, i % 2
        out[b, lh * LH:(lh + 1) * LH, :] = res.results[i]["outT"].T
    return out


# revision 24
# speedup vs baseline: 1.0389x; 1.0242x over previous
"""AdaLN cross-attention + FFN block on 8 TRN2 NeuronCores.

Sharding: 8 cores = 4 batches x 2 L-halves (512 rows each). No collectives:
K/V projections are duplicated across the pair of cores sharing a batch,
everything else splits cleanly along L.

Layout: the whole kernel runs TRANSPOSED — activations are [C, L] with the
channel dim on partitions. This makes every matmul natural (contraction dim
on partitions), makes the AdaLN scale/shift/gamma per-partition broadcasts,
and costs zero on-device transposes. The host supplies x^T, context^T,
exp(bias)^T (per-head [m, l]) and pre-transposed weights; the output comes
back as out^T and is transposed on host.

dtypes: the whole attention path (QKV projections, scores, attn*V, output
projection) runs in fp8-e4m3 with DoubleRow perf mode where the layout
allows (2 k-tiles contracted per pass = 2x PE throughput); weights are
pre-scaled by 16 on the host so fp8 values stay in the normal range, and the
inverse scales fold into activation-scale immediates. The FFN keeps bf16
matmuls (fp8 there pushes rel-err past the gate; measured by numpy
simulation). LayerNorm statistics and residuals stay f32.

The emission order is software-pipelined: V-projection runs first on PE
(needs only ctx+Wv DMA) while DVE/ACT stage the LN1 statistics, and the
attention loop runs with a two-iteration skew — scores for head-pair i,
attention*V for pair i-1, normalization for pair i-2 emitted together.
"""
import sys
if "/opt/trn_rl_repo" not in sys.path:
    sys.path.insert(0, "/opt/trn_rl_repo")

import numpy as np
import ml_dtypes

import concourse.bass as bass
import concourse.mybir as mybir
import concourse.tile as tile
from concourse import bacc
from concourse.bass_utils import run_bass_kernel_spmd

B, L, LC, C, H, HD = 4, 1024, 1024, 1024, 16, 64
P = 128
LH = 512                 # L rows per core
CT = C // P              # 8
MT = LC // P             # 8
E = 4 * C                # 4096
ET = E // P              # 32
SCALE = 0.25 / (HD ** 0.5)
EPS = 1e-5
WS = 16.0                # host-side fp8 weight scale (keeps values normal)
QSC = 1.0 / 32.0         # qT = q_psum * QSC when writing fp8
ESC = SCALE / (WS * WS * QSC)   # exp() scale: scores_psum -> true logits
OSC = 1.0 / (WS * WS)    # out-proj psum -> true attn@Wo

F32 = mybir.dt.float32
F32R = mybir.dt.float32r
BF16 = mybir.dt.bfloat16
F8 = mybir.dt.float8e4
AF = mybir.ActivationFunctionType
ALU = mybir.AluOpType
DR = mybir.MatmulPerfMode.DoubleRow

NCORES = 8


def build():
    nc = bacc.Bacc("TRN2", target_bir_lowering=False, debug=False, num_devices=NCORES)

    xT_d = nc.declare_dram_parameter("xT", [C, LH], F32, isOutput=False)
    ctxT_d = nc.declare_dram_parameter("ctxT", [C, LC], F8, isOutput=False)
    biasT_d = nc.declare_dram_parameter("biasT", [H, LC, LH], F8, isOutput=False)
    wqT_d = nc.declare_dram_parameter("wqT", [P, CT, CT, P], F8, isOutput=False)
    wkT_d = nc.declare_dram_parameter("wkT", [P, CT, CT, P], F8, isOutput=False)
    wvT_d = nc.declare_dram_parameter("wvT", [C, C], F8, isOutput=False)
    woT_d = nc.declare_dram_parameter("woT", [P, CT, CT, P], F8, isOutput=False)
    w1T_d = nc.declare_dram_parameter("w1T", [P, ET, CT, P], BF16, isOutput=False)
    w2T_d = nc.declare_dram_parameter("w2T", [P, CT, 2, 16, P], BF16, isOutput=False)
    bo_d = nc.declare_dram_parameter("bo", [P, CT], F32, isOutput=False)
    b1_d = nc.declare_dram_parameter("b1", [P, ET], F32, isOutput=False)
    b2_d = nc.declare_dram_parameter("b2", [P, CT], F32, isOutput=False)
    cond_d = nc.declare_dram_parameter("cond", [P, 6, CT], F32, isOutput=False)
    ada_d = nc.declare_dram_parameter("ada", [P, 6, CT], F32, isOutput=False)
    outT_d = nc.declare_dram_parameter("outT", [C, LH], F32, isOutput=True)

    with tile.TileContext(nc) as tc:
        with (
            tc.tile_pool(name="cn", bufs=1) as cn,
            tc.tile_pool(name="p1", bufs=1) as p1,
            tc.tile_pool(name="p2", bufs=2) as p2,
            tc.tile_pool(name="p3", bufs=3) as p3,
            tc.tile_pool(name="p4", bufs=4) as p4,
            tc.tile_pool(name="pp", bufs=4) as pp,
            tc.tile_pool(name="rw2", bufs=2) as rw2,
            tc.tile_pool(name="psS", bufs=3, space="PSUM") as psS,      # [P,1024] universal
            tc.tile_pool(name="psA", bufs=2, space="PSUM") as psA,      # [P,512] av/stats/bcast
        ):
            # ---------- constants / params ----------
            ones_f = cn.tile([P, 1], F32, tag="ones_f")
            nc.vector.memset(ones_f[:], 1.0)
            ones_col = cn.tile([P, 1], BF16, tag="ones_col")        # lhsT [K=P, M=1]
            nc.scalar.copy(ones_col[:], ones_f[:])
            ones_rf = cn.tile([1, P], F32, tag="ones_rf")
            nc.vector.memset(ones_rf[:], 1.0)
            ones_row = cn.tile([1, P], F32R, tag="ones_row")        # lhsT [K=1, M=P]
            nc.scalar.copy(ones_row[:], ones_rf[:])
            eps_t = cn.tile([1, 1], F32, tag="eps")
            nc.vector.memset(eps_t[:], EPS)

            # ---------- big persistent tensors ----------
            ctxT = p1.tile([P, CT, LC], F8, tag="ctx")
            wv = p1.tile([P, CT, C], F8, tag="wv")
            xT = p1.tile([P, CT, LH], F32, tag="xT")
            qT = p1.tile([P, CT, LH], F8, tag="qT")
            vaug = p1.tile([P, MT, H, HD + 1], F8, tag="vaug")
            nc.vector.memset(vaug[:, :, :, HD:HD + 1], 1.0)         # softmax-denominator ones column

            # xT tiles lead the queue so LN1 staging starts immediately;
            # ctx + Wv stream concurrently on other queues for the V matmuls
            for ct in range(CT):
                nc.sync.dma_start(xT[:, ct, :], xT_d[ct * P:(ct + 1) * P, :])
            # ctx/Wv split so the first V-projection groups start ~4us earlier
            nc.gpsimd.dma_start(ctxT[:, :, 0:256],
                                ctxT_d[:, 0:256].rearrange("(o p) f -> p o f", p=P))
            nc.scalar.dma_start(wv[:, :, 0:512],
                                wvT_d[:, 0:512].rearrange("(o p) d -> p o d", p=P))
            nc.gpsimd.dma_start(ctxT[:, :, 256:LC],
                                ctxT_d[:, 256:LC].rearrange("(o p) f -> p o f", p=P))
            nc.scalar.dma_start(wv[:, :, 512:C],
                                wvT_d[:, 512:C].rearrange("(o p) d -> p o d", p=P))

            # ---------- AdaLN parameters ----------
            cond_t = cn.tile([P, 6, CT], F32, tag="cond")
            nc.sync.dma_start(cond_t[:], cond_d[:, :, :])
            ada_t = cn.tile([P, 6, CT], F32, tag="ada")
            nc.sync.dma_start(ada_t[:], ada_d[:, :, :])
            g_t = cn.tile([P, 6, CT], F32, tag="g")                 # gamma1,gamma2,scale1,scale2,shift1,shift2
            nc.vector.tensor_tensor(g_t[:], cond_t[:], ada_t[:], ALU.add)
            s1p1 = cn.tile([P, CT], F32, tag="s1p1")                # scale1 + 1
            nc.vector.tensor_scalar_add(s1p1[:], g_t[:, 2, :], 1.0)
            s2p1 = cn.tile([P, CT], F32, tag="s2p1")                # scale2 + 1
            nc.vector.tensor_scalar_add(s2p1[:], g_t[:, 3, :], 1.0)
            g1s = cn.tile([P, CT], F32, tag="g1s")                  # gamma1 / (WS*WS)
            nc.vector.tensor_scalar_mul(g1s[:], g_t[:, 0, :], OSC)
            bo_t = cn.tile([P, CT], F32, tag="bo")
            nc.sync.dma_start(bo_t[:], bo_d[:, :])
            b1_t = cn.tile([P, ET], F32, tag="b1")
            nc.sync.dma_start(b1_t[:], b1_d[:, :])
            b2_t = cn.tile([P, CT], F32, tag="b2")
            nc.sync.dma_start(b2_t[:], b2_d[:, :])
            bog1 = cn.tile([P, CT], F32, tag="bog1")                # bo * gamma1
            nc.vector.tensor_tensor(bog1[:], bo_t[:], g_t[:, 0, :], ALU.mult)
            b2g2 = cn.tile([P, CT], F32, tag="b2g2")                # b2 * gamma2
            nc.vector.tensor_tensor(b2g2[:], b2_t[:], g_t[:, 1, :], ALU.mult)

            # ---------- LN pieces ----------
            def ln_stage(src, ct):
                xr = p4.tile([P, LH], BF16, tag="tmpA")
                nc.vector.tensor_copy(xr[:], src[:, ct, :])
                sq = p4.tile([P, LH], BF16, tag="tmpB")
                nc.scalar.activation(sq[:], src[:, ct, :], AF.Square, bias=0.0, scale=1.0)
                return xr, sq

            def ln_mm(xsum, ssum, xr, sq, ct):
                nc.tensor.matmul(xsum[:], ones_col[:], xr[:], start=(ct == 0), stop=(ct == CT - 1))
                nc.tensor.matmul(ssum[:], ones_col[:], sq[:], start=(ct == 0), stop=(ct == CT - 1))

            def ln_rows(xsum, ssum):
                mu = cn.tile([1, LH], F32, tag="mu")
                nc.vector.tensor_scalar_mul(mu[:], xsum[:], 1.0 / C)
                ex2 = p3.tile([1, LH], F32, tag="rowtmp")
                nc.vector.tensor_scalar_mul(ex2[:], ssum[:], 1.0 / C)
                mu2 = p3.tile([1, LH], F32, tag="rowtmp")
                nc.vector.tensor_tensor(mu2[:], mu[:], mu[:], ALU.mult)
                var = p3.tile([1, LH], F32, tag="rowtmp")
                nc.vector.tensor_tensor(var[:], ex2[:], mu2[:], ALU.subtract)
                sd = p3.tile([1, LH], F32, tag="rowtmp")
                nc.scalar.activation(sd[:], var[:], AF.Sqrt, bias=eps_t[:, 0:1], scale=1.0)
                rstd = cn.tile([1, LH], F32, tag="rstd")
                nc.vector.reciprocal_approx_fast(rstd[:], sd[:])
                rstd_r = cn.tile([1, LH], F32R, tag="rstd_r")
                nc.scalar.copy(rstd_r[:], rstd[:])
                nmr = p3.tile([1, LH], F32, tag="rowtmp")
                nc.vector.tensor_tensor(nmr[:], mu[:], rstd[:], ALU.mult)
                nmr_r = cn.tile([1, LH], F32R, tag="nmr_r")
                nc.scalar.mul(nmr_r[:], nmr[:], -1.0)               # -(mu*rstd)
                return rstd_r, nmr_r

            def ln_bc(rstd_r, nmr_r):
                bc_rp = psA.tile([P, LH], F32, tag="avp")
                nc.tensor.matmul(bc_rp[:], ones_row[:], rstd_r[:], start=True, stop=True)
                bc_r = rw2.tile([P, LH], F32, tag="bcs")
                nc.scalar.copy(bc_r[:], bc_rp[:])
                bc_np = psA.tile([P, LH], F32, tag="avp")
                nc.tensor.matmul(bc_np[:], ones_row[:], nmr_r[:], start=True, stop=True)
                bc_n = rw2.tile([P, LH], F32, tag="bcs")
                nc.scalar.copy(bc_n[:], bc_np[:])
                return bc_r, bc_n

            def ln_apply(src, bc_r, bc_n, sc_col, sh_idx, out_mod):
                for ct in range(CT):
                    t1 = p4.tile([P, LH], F32, tag="tmpA")
                    nc.vector.tensor_tensor(t1[:], src[:, ct, :], bc_r[:], ALU.mult)
                    t2 = p4.tile([P, LH], F32, tag="tmpB")
                    nc.vector.tensor_tensor(t2[:], t1[:], bc_n[:], ALU.add)
                    nc.scalar.activation(out_mod[:, ct, :], t2[:], AF.Identity,
                                         bias=g_t[:, sh_idx, ct:ct + 1],
                                         scale=sc_col[:, ct:ct + 1])

            # ---------- V projection (fp8 DoubleRow over ct pairs) ----------
            def v_group(mt, half):
                v_ps = psS.tile([P, 512], F32, tag="accS")
                for i in range(CT // 2):
                    nc.tensor.matmul(v_ps[:], ctxT[:, 2 * i:2 * i + 2, mt * P:(mt + 1) * P],
                                     wv[:, 2 * i:2 * i + 2, half * 512:(half + 1) * 512],
                                     start=(i == 0), stop=(i == CT // 2 - 1), perf_mode=DR)
                nc.scalar.copy(vaug[:, mt, half * 8:(half + 1) * 8, 0:HD],
                               v_ps[:].rearrange("p (h d) -> p h d", d=HD))

            # LN1 staging (DVE/ACT) runs while PE does the V projection;
            # stats matmuls are interleaved so the accumulation finishes early
            stage = [ln_stage(xT, ct) for ct in range(2)]
            xsum1 = psA.tile([1, LH], F32, tag="avp")
            ssum1 = psA.tile([1, LH], F32, tag="avp")
            for mt in range(MT):
                v_group(mt, 0)
                if mt < CT:
                    xr, sq = stage[mt % 2]
                    ln_mm(xsum1, ssum1, xr, sq, mt)
                    if mt + 2 < CT:
                        stage[mt % 2] = ln_stage(xT, mt + 2)
            r1, n1 = ln_rows(xsum1, ssum1)
            bc_r1, bc_n1 = ln_bc(r1, n1)
            for mt in range(MT):
                v_group(mt, 1)
            modx = p1.tile([P, CT, LH], F8, tag="mod8")
            ln_apply(xT, bc_r1, bc_n1, s1p1, 4, modx)

            # ---------- K projection (fp8 DoubleRow) ----------
            def k_mm(dt):
                wk_st = p4.tile([P, CT, P], F8, tag="wst")
                nc.sync.dma_start(wk_st[:], wkT_d[:, dt, :, :])
                k_ps = psS.tile([P, LC], F32, tag="accS")
                for i in range(CT // 2):
                    nc.tensor.matmul(k_ps[:, 0:512], wk_st[:, 2 * i:2 * i + 2, :],
                                     ctxT[:, 2 * i:2 * i + 2, 0:512],
                                     start=(i == 0), stop=(i == CT // 2 - 1), perf_mode=DR)
                    nc.tensor.matmul(k_ps[:, 512:1024], wk_st[:, 2 * i:2 * i + 2, :],
                                     ctxT[:, 2 * i:2 * i + 2, 512:1024],
                                     start=(i == 0), stop=(i == CT // 2 - 1), perf_mode=DR)
                return k_ps

            def k_copy(k_ps):
                kdt = p2.tile([P, LC], F8, tag="kdt")
                nc.vector.tensor_copy(kdt[:], k_ps[:])
                return kdt

            # K0 runs on PE while ACT produces modx for the Q projection
            kd = {0: k_mm(0)}

            def q_proj(dt):
                wq_st = p4.tile([P, CT, P], F8, tag="wst")
                nc.sync.dma_start(wq_st[:], wqT_d[:, dt, :, :])
                q_ps = psS.tile([P, LH], F32, tag="accS")
                for i in range(CT // 2):
                    nc.tensor.matmul(q_ps[:], wq_st[:, 2 * i:2 * i + 2, :],
                                     modx[:, 2 * i:2 * i + 2, :],
                                     start=(i == 0), stop=(i == CT // 2 - 1), perf_mode=DR)
                nc.scalar.mul(qT[:, dt, :], q_ps[:], QSC)

            kd[0] = k_copy(kd[0])
            # K1 also before Q: fills PE while ACT produces modx, and lightens
            # the first attention-loop iteration
            kd[1] = k_copy(k_mm(1))
            for dt in range(CT):
                q_proj(dt)

            # ---------- attention: two-iteration software pipeline ----------
            # iteration i emits: finish(i-2) | k(i+1) | scores+softmax(i) | attnv(i-1)
            cat = p1.tile([P, CT, LH], F8, tag="cat")               # out^T of attention, head-concat

            def emit_scores(dt):
                kcur = kd[dt]
                out = []
                for hh in range(2):
                    h = 2 * dt + hh
                    probs = pp.tile([P, MT, LH], F8, tag="probs")
                    out.append(probs)
                    for mp in range(MT // 2):           # pairs of m-tiles
                        sc = psS.tile([P, LC], F32, tag="accS")
                        for j in range(2):
                            mt = 2 * mp + j
                            nc.tensor.matmul(
                                sc[:, j * 512:(j + 1) * 512],
                                kcur[hh * HD:(hh + 1) * HD, mt * P:(mt + 1) * P],
                                qT[hh * HD:(hh + 1) * HD, dt, :],
                                start=True, stop=True)
                        # exp(s + b) = exp(s)*exp(b); host ships exp(bias) in fp8
                        bias_t = p4.tile([P, 2, LH], F8, tag="biast")
                        nc.gpsimd.dma_start(
                            bias_t[:], biasT_d[h, 2 * mp * P:(2 * mp + 2) * P, :]
                            .rearrange("(t p) l -> p t l", p=P))
                        es = p3.tile([P, 2, LH], F8, tag="es")
                        nc.scalar.activation(es[:], sc[:].rearrange("p (t l) -> p t l", t=2),
                                             AF.Exp, bias=0.0, scale=ESC)
                        nc.vector.tensor_tensor(probs[:, 2 * mp:2 * mp + 2, :], es[:],
                                                bias_t[:], ALU.mult)
                return out

            def emit_attnv(dt, probs2):
                out = []
                for hh in range(2):
                    h = 2 * dt + hh
                    probs = probs2[hh]
                    av = psA.tile([P, LH], F32, tag="avp")
                    out.append((dt, hh, av))
                    for mp in range(MT // 2):
                        nc.tensor.matmul(av[0:HD + 1, :], vaug[:, 2 * mp:2 * mp + 2, h, :],
                                         probs[:, 2 * mp:2 * mp + 2, :],
                                         start=(mp == 0), stop=(mp == MT // 2 - 1),
                                         perf_mode=DR)
                return out

            def emit_recip(pend_av):
                # all-DVE reciprocal chain, emitted at the start of the NEXT
                # iteration so it runs while PE does the K projection
                out = []
                for (dt, hh, av) in pend_av:
                    ssr = p3.tile([1, LH], F32, tag="rowtmp")
                    nc.scalar.copy(ssr[:], av[HD:HD + 1, :])
                    rec = p3.tile([1, LH], F32, tag="rowtmp")
                    nc.vector.reciprocal_approx_fast(rec[:], ssr[:])
                    rec_r = p3.tile([1, LH], F32R, tag="rowtmp")
                    nc.scalar.copy(rec_r[:], rec[:])
                    out.append((dt, hh, av, rec_r))
                return out

            def head_finish(pdt, phh, av, rec_r):
                bc_ps = psS.tile([P, LH], F32, tag="accS")
                nc.tensor.matmul(bc_ps[0:HD, :], ones_row[:, 0:HD], rec_r[:],
                                 start=True, stop=True)
                bc_s = p4.tile([HD, LH], F32, tag="tmpA")
                nc.scalar.copy(bc_s[:], bc_ps[0:HD, :])
                if phh == 0:
                    nc.vector.tensor_tensor(cat[0:HD, pdt, :], av[0:HD, :], bc_s[:], ALU.mult)
                else:
                    tmp_o = p4.tile([HD, LH], F8, tag="tmpB")
                    nc.vector.tensor_tensor(tmp_o[:], av[0:HD, :], bc_s[:], ALU.mult)
                    nc.sync.dma_start(cat[HD:P, pdt, :], tmp_o[:])   # partition shift

            probs_by_dt = {}
            pend_fin = []   # (dt, hh, av, rec_r) for head_finish one iteration later
            for i in range(CT + 2):
                kp = None
                if i + 1 < CT and (i + 1) not in kd:
                    kp = k_mm(i + 1)
                for args in pend_fin:
                    head_finish(*args)              # bc + normalization for pair i-1
                pend_fin = []
                if kp is not None:
                    kd[i + 1] = k_copy(kp)          # DVE copy before the probs mults
                av_out = []
                if i < CT:
                    probs_by_dt[i] = emit_scores(i)
                if 1 <= i <= CT:
                    av_out = emit_attnv(i - 1, probs_by_dt.pop(i - 1))
                pend_fin = emit_recip(av_out)       # DVE/ACT chain, runs early next iter
            for args in pend_fin:
                head_finish(*args)

            # ---------- output projection + gated residual, LN2 stats interleaved ----------
            xsum2 = psA.tile([1, LH], F32, tag="avp")
            ssum2 = psA.tile([1, LH], F32, tag="avp")
            for ct2 in range(CT):
                wo_st = p4.tile([P, CT, P], F8, tag="wst")
                nc.sync.dma_start(wo_st[:], woT_d[:, ct2, :, :])
                ao_ps = psS.tile([P, LH], F32, tag="accS")
                for i in range(CT // 2):
                    nc.tensor.matmul(ao_ps[:], wo_st[:, 2 * i:2 * i + 2, :],
                                     cat[:, 2 * i:2 * i + 2, :],
                                     start=(i == 0), stop=(i == CT // 2 - 1), perf_mode=DR)
                t = p4.tile([P, LH], F32, tag="tmpB")
                nc.scalar.activation(t[:], ao_ps[:], AF.Identity,
                                     bias=bog1[:, ct2:ct2 + 1], scale=g1s[:, ct2:ct2 + 1])
                nc.vector.tensor_tensor(xT[:, ct2, :], t[:], xT[:, ct2, :], ALU.add)
                xr, sq = ln_stage(xT, ct2)
                ln_mm(xsum2, ssum2, xr, sq, ct2)

            r2, n2 = ln_rows(xsum2, ssum2)
            bc_r2, bc_n2 = ln_bc(r2, n2)
            modf = p1.tile([P, CT, LH], BF16, tag="mod")
            ln_apply(xT, bc_r2, bc_n2, s2p1, 5, modf)

            # ---------- FFN (bf16: fp8 fails the accuracy gate here) ----------
            hT = p1.tile([P, ET, LH], BF16, tag="bigA")             # reuses ctxT slot
            for et in range(ET):
                w1_st = p4.tile([P, CT, P], BF16, tag="wst")
                (nc.sync if et % 2 == 0 else nc.gpsimd).dma_start(w1_st[:], w1T_d[:, et, :, :])
                h_ps = psS.tile([P, LH], F32, tag="accS")
                for ct in range(CT):
                    nc.tensor.matmul(h_ps[:], w1_st[:, ct, :], modf[:, ct, :],
                                     start=(ct == 0), stop=(ct == CT - 1))
                nc.scalar.activation(hT[:, et, :], h_ps[:], AF.Gelu_apprx_tanh,
                                     bias=b1_t[:, et:et + 1], scale=1.0)

            for ct2 in range(CT):
                f_ps = psS.tile([P, LH], F32, tag="accS")
                for eh in range(2):
                    w2_st = p2.tile([P, 16, P], BF16, tag="w2st")
                    (nc.sync if eh == 0 else nc.gpsimd).dma_start(w2_st[:], w2T_d[:, ct2, eh, :, :])
                    for ei in range(16):
                        et = eh * 16 + ei
                        nc.tensor.matmul(f_ps[:], w2_st[:, ei, :], hT[:, et, :],
                                         start=(et == 0), stop=(et == ET - 1))
                t = p4.tile([P, LH], F32, tag="tmpB")
                nc.scalar.activation(t[:], f_ps[:], AF.Identity,
                                     bias=b2g2[:, ct2:ct2 + 1], scale=g_t[:, 1, ct2:ct2 + 1])
                o_t = p4.tile([P, LH], F32, tag="tmpA")
                nc.vector.tensor_tensor(o_t[:], t[:], xT[:, ct2, :], ALU.add)
                nc.sync.dma_start(outT_d[ct2 * P:(ct2 + 1) * P, :], o_t[:])

    nc.compile()
    return nc


_NC = None


def _get_nc():
    global _NC
    if _NC is None:
        _NC = build()
    return _NC


def _shard(inputs):
    f32 = lambda a: np.ascontiguousarray(a, dtype=np.float32)
    bf16 = ml_dtypes.bfloat16
    f8 = ml_dtypes.float8_e4m3
    x = f32(inputs["x"]); context = f32(inputs["context"])
    cond_BD = f32(inputs["cond_BD"]); attn_bias = f32(inputs["attn_bias"])
    ada_gss = f32(inputs["ada_gss"])
    Wq = f32(inputs["Wq"]); Wk = f32(inputs["Wk"]); Wv = f32(inputs["Wv"])
    Wo = f32(inputs["Wo"]); bo = f32(inputs["bo"])
    W1 = f32(inputs["W1"]); b1 = f32(inputs["b1"])
    W2 = f32(inputs["W2"]); b2 = f32(inputs["b2"])

    shared = {
        "wqT": np.ascontiguousarray(
            (WS * Wq).T.reshape(CT, P, CT, P).transpose(1, 2, 0, 3)).astype(f8),
        "wkT": np.ascontiguousarray(
            (WS * Wk).T.reshape(CT, P, CT, P).transpose(1, 2, 0, 3)).astype(f8),
        "wvT": np.ascontiguousarray((WS * Wv).T).astype(f8),
        "woT": np.ascontiguousarray(
            (WS * Wo).T.reshape(CT, P, CT, P).transpose(1, 2, 0, 3)).astype(f8),
        "w1T": np.ascontiguousarray(
            W1.T.reshape(CT, P, ET, P).transpose(1, 2, 0, 3)).astype(bf16),
        "w2T": np.ascontiguousarray(
            W2.T.reshape(2, 16, P, CT, P).transpose(2, 3, 0, 1, 4)).astype(bf16),
        "bo": np.ascontiguousarray(bo.reshape(CT, P).T),
        "b1": np.ascontiguousarray(b1.reshape(ET, P).T),
        "b2": np.ascontiguousarray(b2.reshape(CT, P).T),
        "ada": np.ascontiguousarray(ada_gss[0, 0].reshape(6, CT, P).transpose(2, 0, 1)),
    }
    in_maps = []
    for i in range(NCORES):
        b, lh = i // 2, i % 2
        l0 = lh * LH
        m = dict(shared)
        m["xT"] = np.ascontiguousarray(x[b, l0:l0 + LH, :].T)
        m["ctxT"] = np.ascontiguousarray(context[b].T).astype(f8)
        m["biasT"] = np.exp(np.ascontiguousarray(
            attn_bias[b, :, l0:l0 + LH, :].transpose(0, 2, 1))).astype(f8)
        m["cond"] = np.ascontiguousarray(cond_BD[b, 0].reshape(6, CT, P).transpose(2, 0, 1))
        in_maps.append(m)
    return in_maps


def kernel(**inputs) -> np.ndarray:
    nc = _get_nc()
    in_maps = _shard(inputs)
    res = run_bass_kernel_spmd(nc, in_maps, core_ids=list(range(NCORES)))
    out = np.empty((B, L, C), dtype=np.float32)
    for i in range(NCORES):
        b, lh = i // 2, i % 2
        out[b, lh * LH:(lh + 1) * LH, :] = res.results[i]["outT"].T
    return out


# revision 30
# speedup vs baseline: 1.0683x; 1.0283x over previous
"""AdaLN cross-attention + FFN block on 8 TRN2 NeuronCores.

Sharding: 8 cores = 4 batches x 2 L-halves (512 rows each). No collectives:
K/V projections are duplicated across the pair of cores sharing a batch,
everything else splits cleanly along L.

Layout: the whole kernel runs TRANSPOSED — activations are [C, L] with the
channel dim on partitions. This makes every matmul natural (contraction dim
on partitions), makes the AdaLN scale/shift/gamma per-partition broadcasts,
and costs zero on-device transposes. The host supplies x^T, context^T,
exp(bias)^T (per-head [m, l]) and pre-transposed weights; the output comes
back as out^T and is transposed on host.

dtypes: the whole attention path (QKV projections, scores, attn*V, output
projection) runs in fp8-e4m3 with DoubleRow perf mode where the layout
allows (2 k-tiles contracted per pass = 2x PE throughput); weights are
pre-scaled by 16 on the host so fp8 values stay in the normal range, and the
inverse scales fold into activation-scale immediates. The FFN keeps bf16
matmuls (fp8 there pushes rel-err past the gate; measured by numpy
simulation). LayerNorm statistics and residuals stay f32.

The emission order is software-pipelined: V-projection runs first on PE
(needs only ctx+Wv DMA) while DVE/ACT stage the LN1 statistics, and the
attention loop runs with a two-iteration skew — scores for head-pair i,
attention*V for pair i-1, normalization for pair i-2 emitted together.
"""
import sys
if "/opt/trn_rl_repo" not in sys.path:
    sys.path.insert(0, "/opt/trn_rl_repo")

import numpy as np
import ml_dtypes

import concourse.bass as bass
import concourse.mybir as mybir
import concourse.tile as tile
from concourse import bacc
from concourse.bass_utils import run_bass_kernel_spmd

B, L, LC, C, H, HD = 4, 1024, 1024, 1024, 16, 64
P = 128
LH = 512                 # L rows per core
CT = C // P              # 8
MT = LC // P             # 8
E = 4 * C                # 4096
ET = E // P              # 32
SCALE = 0.25 / (HD ** 0.5)
EPS = 1e-5
WS = 16.0                # host-side fp8 weight scale (keeps values normal)
QSC = 1.0 / 32.0         # qT = q_psum * QSC when writing fp8
ESC = SCALE / (WS * WS * QSC)   # exp() scale: scores_psum -> true logits
OSC = 1.0 / (WS * WS)    # out-proj psum -> true attn@Wo

F32 = mybir.dt.float32
F32R = mybir.dt.float32r
BF16 = mybir.dt.bfloat16
F8 = mybir.dt.float8e4
AF = mybir.ActivationFunctionType
ALU = mybir.AluOpType
DR = mybir.MatmulPerfMode.DoubleRow

NCORES = 8


def build():
    nc = bacc.Bacc("TRN2", target_bir_lowering=False, debug=False, num_devices=NCORES)

    xT_d = nc.declare_dram_parameter("xT", [C, LH], F32, isOutput=False)
    xT8_d = nc.declare_dram_parameter("xT8", [C, LH], BF16, isOutput=False)
    ctxT_d = nc.declare_dram_parameter("ctxT", [C, LC], F8, isOutput=False)
    biasT_d = nc.declare_dram_parameter("biasT", [H, LC, LH], F8, isOutput=False)
    wqT_d = nc.declare_dram_parameter("wqT", [P, CT, CT, P], F8, isOutput=False)
    wkT_d = nc.declare_dram_parameter("wkT", [P, CT, CT, P], F8, isOutput=False)
    wvT_d = nc.declare_dram_parameter("wvT", [C, C], F8, isOutput=False)
    woT_d = nc.declare_dram_parameter("woT", [P, CT, CT, P], F8, isOutput=False)
    w1T_d = nc.declare_dram_parameter("w1T", [P, ET, CT, P], BF16, isOutput=False)
    w2T_d = nc.declare_dram_parameter("w2T", [P, CT, 2, 16, P], BF16, isOutput=False)
    bo_d = nc.declare_dram_parameter("bo", [P, CT], F32, isOutput=False)
    b1_d = nc.declare_dram_parameter("b1", [P, ET], F32, isOutput=False)
    b2_d = nc.declare_dram_parameter("b2", [P, CT], F32, isOutput=False)
    cond_d = nc.declare_dram_parameter("cond", [P, 6, CT], F32, isOutput=False)
    ada_d = nc.declare_dram_parameter("ada", [P, 6, CT], F32, isOutput=False)
    outT_d = nc.declare_dram_parameter("outT", [C, LH], F32, isOutput=True)

    with tile.TileContext(nc) as tc:
        with (
            tc.tile_pool(name="cn", bufs=1) as cn,
            tc.tile_pool(name="p1", bufs=1) as p1,
            tc.tile_pool(name="p2", bufs=2) as p2,
            tc.tile_pool(name="p3", bufs=3) as p3,
            tc.tile_pool(name="p4", bufs=4) as p4,
            tc.tile_pool(name="pp", bufs=4) as pp,
            tc.tile_pool(name="rw2", bufs=2) as rw2,
            tc.tile_pool(name="psS", bufs=3, space="PSUM") as psS,      # [P,1024] universal
            tc.tile_pool(name="psA", bufs=2, space="PSUM") as psA,      # [P,512] av/stats/bcast
        ):
            # ---------- constants / params ----------
            ones_f = cn.tile([P, 1], F32, tag="ones_f")
            nc.vector.memset(ones_f[:], 1.0)
            ones_col = cn.tile([P, 1], BF16, tag="ones_col")        # lhsT [K=P, M=1]
            nc.scalar.copy(ones_col[:], ones_f[:])
            ones_rf = cn.tile([1, P], F32, tag="ones_rf")
            nc.vector.memset(ones_rf[:], 1.0)
            ones_row = cn.tile([1, P], F32R, tag="ones_row")        # lhsT [K=1, M=P]
            nc.scalar.copy(ones_row[:], ones_rf[:])
            eps_t = cn.tile([1, 1], F32, tag="eps")
            nc.vector.memset(eps_t[:], EPS)

            # ---------- big persistent tensors ----------
            ctxT = p1.tile([P, CT, LC], F8, tag="ctx")
            wv = p1.tile([P, CT, C], F8, tag="wv")
            xT = p1.tile([P, CT, LH], F32, tag="xT")
            xT8 = p1.tile([P, CT, LH], BF16, tag="xT8")
            qT = p1.tile([P, CT, LH], F8, tag="qT")
            vaug = p1.tile([P, MT, H, HD + 1], F8, tag="vaug")
            nc.vector.memset(vaug[:, :, :, HD:HD + 1], 1.0)         # softmax-denominator ones column

            # xT tiles lead the queue so LN1 staging starts immediately;
            # ctx + Wv stream concurrently on other queues for the V matmuls
            for ct in range(CT):
                nc.sync.dma_start(xT8[:, ct, :], xT8_d[ct * P:(ct + 1) * P, :])
            # ctx/Wv split so the first V-projection groups start ~4us earlier
            nc.gpsimd.dma_start(ctxT[:, :, 0:256],
                                ctxT_d[:, 0:256].rearrange("(o p) f -> p o f", p=P))
            nc.scalar.dma_start(wv[:, :, 0:512],
                                wvT_d[:, 0:512].rearrange("(o p) d -> p o d", p=P))
            nc.gpsimd.dma_start(ctxT[:, :, 256:LC],
                                ctxT_d[:, 256:LC].rearrange("(o p) f -> p o f", p=P))
            nc.scalar.dma_start(wv[:, :, 512:C],
                                wvT_d[:, 512:C].rearrange("(o p) d -> p o d", p=P))

            # ---------- AdaLN parameters ----------
            cond_t = cn.tile([P, 6, CT], F32, tag="cond")
            nc.sync.dma_start(cond_t[:], cond_d[:, :, :])
            ada_t = cn.tile([P, 6, CT], F32, tag="ada")
            nc.sync.dma_start(ada_t[:], ada_d[:, :, :])
            g_t = cn.tile([P, 6, CT], F32, tag="g")                 # gamma1,gamma2,scale1,scale2,shift1,shift2
            nc.vector.tensor_tensor(g_t[:], cond_t[:], ada_t[:], ALU.add)
            s1p1 = cn.tile([P, CT], F32, tag="s1p1")                # scale1 + 1
            nc.vector.tensor_scalar_add(s1p1[:], g_t[:, 2, :], 1.0)
            s2p1 = cn.tile([P, CT], F32, tag="s2p1")                # scale2 + 1
            nc.vector.tensor_scalar_add(s2p1[:], g_t[:, 3, :], 1.0)
            g1s = cn.tile([P, CT], F32, tag="g1s")                  # gamma1 / (WS*WS)
            nc.vector.tensor_scalar_mul(g1s[:], g_t[:, 0, :], OSC)
            bo_t = cn.tile([P, CT], F32, tag="bo")
            nc.sync.dma_start(bo_t[:], bo_d[:, :])
            b1_t = cn.tile([P, ET], F32, tag="b1")
            nc.sync.dma_start(b1_t[:], b1_d[:, :])
            b2_t = cn.tile([P, CT], F32, tag="b2")
            nc.sync.dma_start(b2_t[:], b2_d[:, :])
            bog1 = cn.tile([P, CT], F32, tag="bog1")                # bo * gamma1
            nc.vector.tensor_tensor(bog1[:], bo_t[:], g_t[:, 0, :], ALU.mult)
            b2g2 = cn.tile([P, CT], F32, tag="b2g2")                # b2 * gamma2
            nc.vector.tensor_tensor(b2g2[:], b2_t[:], g_t[:, 1, :], ALU.mult)

            # ---------- LN pieces ----------
            def ln_stage(src, ct):
                xr = p4.tile([P, LH], BF16, tag="tmpA")
                nc.vector.tensor_copy(xr[:], src[:, ct, :])
                sq = p4.tile([P, LH], BF16, tag="tmpB")
                nc.scalar.activation(sq[:], src[:, ct, :], AF.Square, bias=0.0, scale=1.0)
                return xr, sq

            def ln_mm(xsum, ssum, xr, sq, ct):
                nc.tensor.matmul(xsum[:], ones_col[:], xr[:], start=(ct == 0), stop=(ct == CT - 1))
                nc.tensor.matmul(ssum[:], ones_col[:], sq[:], start=(ct == 0), stop=(ct == CT - 1))

            def ln_rows(xsum, ssum):
                mu = cn.tile([1, LH], F32, tag="mu")
                nc.vector.tensor_scalar_mul(mu[:], xsum[:], 1.0 / C)
                ex2 = p3.tile([1, LH], F32, tag="rowtmp")
                nc.vector.tensor_scalar_mul(ex2[:], ssum[:], 1.0 / C)
                mu2 = p3.tile([1, LH], F32, tag="rowtmp")
                nc.vector.tensor_tensor(mu2[:], mu[:], mu[:], ALU.mult)
                var = p3.tile([1, LH], F32, tag="rowtmp")
                nc.vector.tensor_tensor(var[:], ex2[:], mu2[:], ALU.subtract)
                sd = p3.tile([1, LH], F32, tag="rowtmp")
                nc.scalar.activation(sd[:], var[:], AF.Sqrt, bias=eps_t[:, 0:1], scale=1.0)
                rstd = cn.tile([1, LH], F32, tag="rstd")
                nc.vector.reciprocal_approx_fast(rstd[:], sd[:])
                rstd_r = cn.tile([1, LH], F32R, tag="rstd_r")
                nc.scalar.copy(rstd_r[:], rstd[:])
                nmr = p3.tile([1, LH], F32, tag="rowtmp")
                nc.vector.tensor_tensor(nmr[:], mu[:], rstd[:], ALU.mult)
                nmr_r = cn.tile([1, LH], F32R, tag="nmr_r")
                nc.scalar.mul(nmr_r[:], nmr[:], -1.0)               # -(mu*rstd)
                return rstd_r, nmr_r

            def ln_bc(rstd_r, nmr_r):
                bc_rp = psA.tile([P, LH], F32, tag="avp")
                nc.tensor.matmul(bc_rp[:], ones_row[:], rstd_r[:], start=True, stop=True)
                bc_r = rw2.tile([P, LH], F32, tag="bcs")
                nc.scalar.copy(bc_r[:], bc_rp[:])
                bc_np = psA.tile([P, LH], F32, tag="avp")
                nc.tensor.matmul(bc_np[:], ones_row[:], nmr_r[:], start=True, stop=True)
                bc_n = rw2.tile([P, LH], F32, tag="bcs")
                nc.scalar.copy(bc_n[:], bc_np[:])
                return bc_r, bc_n

            def ln_apply(src, bc_r, bc_n, sc_col, sh_idx, out_mod):
                for ct in range(CT):
                    t1 = p4.tile([P, LH], F32, tag="tmpA")
                    nc.vector.tensor_tensor(t1[:], src[:, ct, :], bc_r[:], ALU.mult)
                    t2 = p4.tile([P, LH], F32, tag="tmpB")
                    nc.vector.tensor_tensor(t2[:], t1[:], bc_n[:], ALU.add)
                    nc.scalar.activation(out_mod[:, ct, :], t2[:], AF.Identity,
                                         bias=g_t[:, sh_idx, ct:ct + 1],
                                         scale=sc_col[:, ct:ct + 1])

            # ---------- V projection (fp8 DoubleRow over ct pairs) ----------
            def v_group(mt, half):
                v_ps = psS.tile([P, 512], F32, tag="accS")
                for i in range(CT // 2):
                    nc.tensor.matmul(v_ps[:], ctxT[:, 2 * i:2 * i + 2, mt * P:(mt + 1) * P],
                                     wv[:, 2 * i:2 * i + 2, half * 512:(half + 1) * 512],
                                     start=(i == 0), stop=(i == CT // 2 - 1), perf_mode=DR)
                nc.scalar.copy(vaug[:, mt, half * 8:(half + 1) * 8, 0:HD],
                               v_ps[:].rearrange("p (h d) -> p h d", d=HD))

            # LN1 staging (DVE/ACT) runs while PE does the V projection;
            # stats matmuls are interleaved so the accumulation finishes early
            # LN1 stats read the bf16 xT8 tiles directly (no staging copy)
            xsum1 = psA.tile([1, LH], F32, tag="avp")
            ssum1 = psA.tile([1, LH], F32, tag="avp")
            sqs = []
            for ct in range(2):
                sq = p4.tile([P, LH], BF16, tag="tmpB")
                nc.scalar.activation(sq[:], xT8[:, ct, :], AF.Square, bias=0.0, scale=1.0)
                sqs.append(sq)
            for mt in range(MT):
                v_group(mt, 0)
                if mt < CT:
                    nc.tensor.matmul(xsum1[:], ones_col[:], xT8[:, mt, :],
                                     start=(mt == 0), stop=(mt == CT - 1))
                    nc.tensor.matmul(ssum1[:], ones_col[:], sqs[mt % 2][:],
                                     start=(mt == 0), stop=(mt == CT - 1))
                    if mt + 2 < CT:
                        sq = p4.tile([P, LH], BF16, tag="tmpB")
                        nc.scalar.activation(sq[:], xT8[:, mt + 2, :], AF.Square,
                                             bias=0.0, scale=1.0)
                        sqs[mt % 2] = sq
            r1, n1 = ln_rows(xsum1, ssum1)
            bc_r1, bc_n1 = ln_bc(r1, n1)
            for mt in range(MT):
                v_group(mt, 1)
            modx = p1.tile([P, CT, LH], F8, tag="mod8")
            ln_apply(xT8, bc_r1, bc_n1, s1p1, 4, modx)

            # ---------- K projection (fp8 DoubleRow) ----------
            def k_mm(dt):
                wk_st = p4.tile([P, CT, P], F8, tag="wst")
                nc.sync.dma_start(wk_st[:], wkT_d[:, dt, :, :])
                k_ps = psS.tile([P, LC], F32, tag="accS")
                for i in range(CT // 2):
                    nc.tensor.matmul(k_ps[:, 0:512], wk_st[:, 2 * i:2 * i + 2, :],
                                     ctxT[:, 2 * i:2 * i + 2, 0:512],
                                     start=(i == 0), stop=(i == CT // 2 - 1), perf_mode=DR)
                    nc.tensor.matmul(k_ps[:, 512:1024], wk_st[:, 2 * i:2 * i + 2, :],
                                     ctxT[:, 2 * i:2 * i + 2, 512:1024],
                                     start=(i == 0), stop=(i == CT // 2 - 1), perf_mode=DR)
                return k_ps

            def k_copy(k_ps):
                kdt = p2.tile([P, LC], F8, tag="kdt")
                nc.vector.tensor_copy(kdt[:], k_ps[:])
                return kdt

            # K0 runs on PE while ACT produces modx for the Q projection
            kd = {0: k_mm(0)}

            def q_proj(dt):
                wq_st = p4.tile([P, CT, P], F8, tag="wst")
                nc.sync.dma_start(wq_st[:], wqT_d[:, dt, :, :])
                q_ps = psS.tile([P, LH], F32, tag="accS")
                for i in range(CT // 2):
                    nc.tensor.matmul(q_ps[:], wq_st[:, 2 * i:2 * i + 2, :],
                                     modx[:, 2 * i:2 * i + 2, :],
                                     start=(i == 0), stop=(i == CT // 2 - 1), perf_mode=DR)
                nc.scalar.mul(qT[:, dt, :], q_ps[:], QSC)

            kd[0] = k_copy(kd[0])
            # K1 also before Q: fills PE while ACT produces modx, and lightens
            # the first attention-loop iteration
            kd[1] = k_copy(k_mm(1))
            for dt in range(CT):
                q_proj(dt)
            # f32 x for the residual: streams during attention, needed ~150us in
            for ct in range(CT):
                nc.sync.dma_start(xT[:, ct, :], xT_d[ct * P:(ct + 1) * P, :])

            # ---------- attention: two-iteration software pipeline ----------
            # iteration i emits: finish(i-2) | k(i+1) | scores+softmax(i) | attnv(i-1)
            cat = p1.tile([P, CT, LH], F8, tag="cat")               # out^T of attention, head-concat

            def emit_scores(dt):
                kcur = kd[dt]
                out = []
                for hh in range(2):
                    h = 2 * dt + hh
                    probs = pp.tile([P, MT, LH], F8, tag="probs")
                    out.append(probs)
                    for mp in range(MT // 2):           # pairs of m-tiles
                        sc = psS.tile([P, LC], F32, tag="accS")
                        for j in range(2):
                            mt = 2 * mp + j
                            nc.tensor.matmul(
                                sc[:, j * 512:(j + 1) * 512],
                                kcur[hh * HD:(hh + 1) * HD, mt * P:(mt + 1) * P],
                                qT[hh * HD:(hh + 1) * HD, dt, :],
                                start=True, stop=True)
                        # exp(s + b) = exp(s)*exp(b); host ships exp(bias) in fp8
                        bias_t = p4.tile([P, 2, LH], F8, tag="biast")
                        nc.gpsimd.dma_start(
                            bias_t[:], biasT_d[h, 2 * mp * P:(2 * mp + 2) * P, :]
                            .rearrange("(t p) l -> p t l", p=P))
                        es = p3.tile([P, 2, LH], F8, tag="es")
                        nc.scalar.activation(es[:], sc[:].rearrange("p (t l) -> p t l", t=2),
                                             AF.Exp, bias=0.0, scale=ESC)
                        nc.vector.tensor_tensor(probs[:, 2 * mp:2 * mp + 2, :], es[:],
                                                bias_t[:], ALU.mult)
                return out

            def emit_attnv(dt, probs2):
                out = []
                for hh in range(2):
                    h = 2 * dt + hh
                    probs = probs2[hh]
                    av = psA.tile([P, LH], F32, tag="avp")
                    out.append((dt, hh, av))
                    for mp in range(MT // 2):
                        nc.tensor.matmul(av[0:HD + 1, :], vaug[:, 2 * mp:2 * mp + 2, h, :],
                                         probs[:, 2 * mp:2 * mp + 2, :],
                                         start=(mp == 0), stop=(mp == MT // 2 - 1),
                                         perf_mode=DR)
                return out

            def emit_recip(pend_av):
                # all-DVE reciprocal chain, emitted at the start of the NEXT
                # iteration so it runs while PE does the K projection
                out = []
                for (dt, hh, av) in pend_av:
                    ssr = p3.tile([1, LH], F32, tag="rowtmp")
                    nc.scalar.copy(ssr[:], av[HD:HD + 1, :])
                    rec = p3.tile([1, LH], F32, tag="rowtmp")
                    nc.vector.reciprocal_approx_fast(rec[:], ssr[:])
                    rec_r = p3.tile([1, LH], F32R, tag="rowtmp")
                    nc.scalar.copy(rec_r[:], rec[:])
                    out.append((dt, hh, av, rec_r))
                return out

            def head_finish(pdt, phh, av, rec_r):
                bc_ps = psS.tile([P, LH], F32, tag="accS")
                nc.tensor.matmul(bc_ps[0:HD, :], ones_row[:, 0:HD], rec_r[:],
                                 start=True, stop=True)
                bc_s = p4.tile([HD, LH], F32, tag="tmpA")
                nc.scalar.copy(bc_s[:], bc_ps[0:HD, :])
                if phh == 0:
                    nc.vector.tensor_tensor(cat[0:HD, pdt, :], av[0:HD, :], bc_s[:], ALU.mult)
                else:
                    tmp_o = p4.tile([HD, LH], F8, tag="tmpB")
                    nc.vector.tensor_tensor(tmp_o[:], av[0:HD, :], bc_s[:], ALU.mult)
                    nc.sync.dma_start(cat[HD:P, pdt, :], tmp_o[:])   # partition shift

            probs_by_dt = {}
            pend_fin = []   # (dt, hh, av, rec_r) for head_finish one iteration later
            for i in range(CT + 2):
                kp = None
                if i + 1 < CT and (i + 1) not in kd:
                    kp = k_mm(i + 1)
                for args in pend_fin:
                    head_finish(*args)              # bc + normalization for pair i-1
                pend_fin = []
                if kp is not None:
                    kd[i + 1] = k_copy(kp)          # DVE copy before the probs mults
                av_out = []
                if i < CT:
                    probs_by_dt[i] = emit_scores(i)
                if 1 <= i <= CT:
                    av_out = emit_attnv(i - 1, probs_by_dt.pop(i - 1))
                pend_fin = emit_recip(av_out)       # DVE/ACT chain, runs early next iter
            for args in pend_fin:
                head_finish(*args)

            # ---------- output projection + gated residual, LN2 stats interleaved ----------
            xsum2 = psA.tile([1, LH], F32, tag="avp")
            ssum2 = psA.tile([1, LH], F32, tag="avp")
            for ct2 in range(CT):
                wo_st = p4.tile([P, CT, P], F8, tag="wst")
                nc.sync.dma_start(wo_st[:], woT_d[:, ct2, :, :])
                ao_ps = psS.tile([P, LH], F32, tag="accS")
                for i in range(CT // 2):
                    nc.tensor.matmul(ao_ps[:], wo_st[:, 2 * i:2 * i + 2, :],
                                     cat[:, 2 * i:2 * i + 2, :],
                                     start=(i == 0), stop=(i == CT // 2 - 1), perf_mode=DR)
                t = p4.tile([P, LH], F32, tag="tmpB")
                nc.scalar.activation(t[:], ao_ps[:], AF.Identity,
                                     bias=bog1[:, ct2:ct2 + 1], scale=g1s[:, ct2:ct2 + 1])
                nc.vector.tensor_tensor(xT[:, ct2, :], t[:], xT[:, ct2, :], ALU.add)
                xr, sq = ln_stage(xT, ct2)
                ln_mm(xsum2, ssum2, xr, sq, ct2)

            r2, n2 = ln_rows(xsum2, ssum2)
            bc_r2, bc_n2 = ln_bc(r2, n2)
            modf = p1.tile([P, CT, LH], BF16, tag="mod")
            ln_apply(xT, bc_r2, bc_n2, s2p1, 5, modf)

            # ---------- FFN (bf16: fp8 fails the accuracy gate here) ----------
            hT = p1.tile([P, ET, LH], BF16, tag="bigA")             # reuses ctxT slot
            for et in range(ET):
                w1_st = p4.tile([P, CT, P], BF16, tag="wst")
                (nc.sync if et % 2 == 0 else nc.gpsimd).dma_start(w1_st[:], w1T_d[:, et, :, :])
                h_ps = psS.tile([P, LH], F32, tag="accS")
                for ct in range(CT):
                    nc.tensor.matmul(h_ps[:], w1_st[:, ct, :], modf[:, ct, :],
                                     start=(ct == 0), stop=(ct == CT - 1))
                nc.scalar.activation(hT[:, et, :], h_ps[:], AF.Gelu_apprx_tanh,
                                     bias=b1_t[:, et:et + 1], scale=1.0)

            for ct2 in range(CT):
                f_ps = psS.tile([P, LH], F32, tag="accS")
                for eh in range(2):
                    w2_st = p2.tile([P, 16, P], BF16, tag="w2st")
                    (nc.sync if eh == 0 else nc.gpsimd).dma_start(w2_st[:], w2T_d[:, ct2, eh, :, :])
                    for ei in range(16):
                        et = eh * 16 + ei
                        nc.tensor.matmul(f_ps[:], w2_st[:, ei, :], hT[:, et, :],
                                         start=(et == 0), stop=(et == ET - 1))
                t = p4.tile([P, LH], F32, tag="tmpB")
                nc.scalar.activation(t[:], f_ps[:], AF.Identity,
                                     bias=b2g2[:, ct2:ct2 + 1], scale=g_t[:, 1, ct2:ct2 + 1])
                o_t = p4.tile([P, LH], F32, tag="tmpA")
                nc.vector.tensor_tensor(o_t[:], t[:], xT[:, ct2, :], ALU.add)
                nc.sync.dma_start(outT_d[ct2 * P:(ct2 + 1) * P, :], o_t[:])

    nc.compile()
    return nc


_NC = None


def _get_nc():
    global _NC
    if _NC is None:
        _NC = build()
    return _NC


def _shard(inputs):
    f32 = lambda a: np.ascontiguousarray(a, dtype=np.float32)
    bf16 = ml_dtypes.bfloat16
    f8 = ml_dtypes.float8_e4m3
    x = f32(inputs["x"]); context = f32(inputs["context"])
    cond_BD = f32(inputs["cond_BD"]); attn_bias = f32(inputs["attn_bias"])
    ada_gss = f32(inputs["ada_gss"])
    Wq = f32(inputs["Wq"]); Wk = f32(inputs["Wk"]); Wv = f32(inputs["Wv"])
    Wo = f32(inputs["Wo"]); bo = f32(inputs["bo"])
    W1 = f32(inputs["W1"]); b1 = f32(inputs["b1"])
    W2 = f32(inputs["W2"]); b2 = f32(inputs["b2"])

    shared = {
        "wqT": np.ascontiguousarray(
            (WS * Wq).T.reshape(CT, P, CT, P).transpose(1, 2, 0, 3)).astype(f8),
        "wkT": np.ascontiguousarray(
            (WS * Wk).T.reshape(CT, P, CT, P).transpose(1, 2, 0, 3)).astype(f8),
        "wvT": np.ascontiguousarray((WS * Wv).T).astype(f8),
        "woT": np.ascontiguousarray(
            (WS * Wo).T.reshape(CT, P, CT, P).transpose(1, 2, 0, 3)).astype(f8),
        "w1T": np.ascontiguousarray(
            W1.T.reshape(CT, P, ET, P).transpose(1, 2, 0, 3)).astype(bf16),
        "w2T": np.ascontiguousarray(
            W2.T.reshape(2, 16, P, CT, P).transpose(2, 3, 0, 1, 4)).astype(bf16),
        "bo": np.ascontiguousarray(bo.reshape(CT, P).T),
        "b1": np.ascontiguousarray(b1.reshape(ET, P).T),
        "b2": np.ascontiguousarray(b2.reshape(CT, P).T),
        "ada": np.ascontiguousarray(ada_gss[0, 0].reshape(6, CT, P).transpose(2, 0, 1)),
    }
    in_maps = []
    for i in range(NCORES):
        b, lh = i // 2, i % 2
        l0 = lh * LH
        m = dict(shared)
        m["xT"] = np.ascontiguousarray(x[b, l0:l0 + LH, :].T)
        m["xT8"] = np.ascontiguousarray(x[b, l0:l0 + LH, :].T).astype(bf16)
        m["ctxT"] = np.ascontiguousarray(context[b].T).astype(f8)
        m["biasT"] = np.exp(np.ascontiguousarray(
            attn_bias[b, :, l0:l0 + LH, :].transpose(0, 2, 1))).astype(f8)
        m["cond"] = np.ascontiguousarray(cond_BD[b, 0].reshape(6, CT, P).transpose(2, 0, 1))
        in_maps.append(m)
    return in_maps


def kernel(**inputs) -> np.ndarray:
    nc = _get_nc()
    in_maps = _shard(inputs)
    res = run_bass_kernel_spmd(nc, in_maps, core_ids=list(range(NCORES)))
    out = np.empty((B, L, C), dtype=np.float32)
    for i in range(NCORES):
        b, lh = i // 2, i % 2
        out[b, lh * LH:(lh + 1) * LH, :] = res.results[i]["outT"].T
    return out
